# revision 1
# baseline (speedup 1.0000x reference)
"""Trainium2 Bass kernel for a 6-layer GCN autoencoder (50k nodes, 800k edges).

Self-contained: host-side graph preprocessing (node permutation/packing,
edge binning, degree norms), an 8-core SPMD Bass/Tile program (feature-major
dataflow, one-hot scatter matmuls, dma_gather halo reads, AllGather
collectives), and output assembly.
"""
import sys
sys.path.insert(0, '/opt/trn_rl_repo')

import contextlib
import ctypes
import os
import types

import numpy as np
import ml_dtypes

import concourse.bacc as bacc
import concourse.bass as bass
import concourse.mybir as mybir
import concourse.tile as tile
from concourse.library_config import mlp
from concourse.vector_clock import ScopedClock
from concourse.bass_utils import run_bass_kernel_spmd


# ---- workaround: this walrus build rejects >2 sync waits on one instruction;
# spread Tile's tail-drain waits across single-wait SP NOPs.
def _patched_drain_and_barrier(self, tick_clock, wait_clock):
    nc = self.nc
    probe = nc.sync.nop()
    wait_clock.add_sem_waits(probe.ins, ScopedClock({None: tick_clock.global_clock}))
    si = probe.ins.sync_info
    waits = list(si.on_wait) if si is not None else []
    if si is not None:
        while si.on_wait:
            si.on_wait.pop()
    for w in waits:
        n = nc.sync.nop()
        n.ins.sync_info = mybir.SyncInfo(on_wait=[w], on_update=[])
    nc.sync.drain()
    nc.all_engine_barrier()
    assert self.sems is not None
    popped = nc._tile_sem_poison_stack.pop()
    assert popped is self._sem_poison
    nc.clear_and_free_semaphores(list(self.sems.allocated().values()))
    nc.all_engine_barrier()


tile.TileContext._drain_and_barrier = _patched_drain_and_barrier


# ---- optional NTFF profiling hook (GCAE_TRACE=1)
def _install_profile_hook():
    try:
        import antenv
    except ImportError:
        return False
    if getattr(antenv, "axon_hooks", None) is not None:
        return True
    so_path = "/opt/axon/libaxon_pjrt.so"
    if not os.path.exists(so_path):
        return False
    lib = ctypes.CDLL(so_path)
    if not hasattr(lib, "axon_start_nrt_profile"):
        return False
    lib.axon_start_nrt_profile.argtypes = [ctypes.POINTER(ctypes.c_int64), ctypes.c_size_t]
    lib.axon_start_nrt_profile.restype = ctypes.c_int64
    lib.axon_stop_nrt_profile.argtypes = [ctypes.c_char_p]
    lib.axon_stop_nrt_profile.restype = ctypes.c_int64

    @contextlib.contextmanager
    def _hook(output_dir, device_ids):
        import jax
        jax.devices()
        if device_ids:
            ids = (ctypes.c_int64 * len(device_ids))(*device_ids)
            rc = lib.axon_start_nrt_profile(ids, len(device_ids))
        else:
            rc = lib.axon_start_nrt_profile(None, 0)
        if rc != 0:
            raise RuntimeError(f"axon_start_nrt_profile rc={rc}")
        try:
            yield
        finally:
            n = lib.axon_stop_nrt_profile(str(output_dir).encode())
            if n < 0:
                raise RuntimeError(f"axon_stop_nrt_profile rc={n}")

    hooks = types.ModuleType("antenv.axon_hooks")
    _h = [_hook]
    hooks.set_axon_ntff_profile_hook = lambda h: _h.__setitem__(0, h)
    hooks.get_axon_ntff_profile_hook = lambda: _h[0]
    sys.modules["antenv.axon_hooks"] = hooks
    antenv.axon_hooks = hooks
    return True

F32 = mybir.dt.float32
BF16 = mybir.dt.bfloat16
I16 = mybir.dt.int16

N = 50000
NC = 8
BLK = 128
BPC = 49                 # blocks per core
NPC = BPC * BLK          # 6272 nodes per core
NPAD = NC * NPC          # 50176
HALF = NPAD // 2         # 25088
NT = BPC                 # node tiles per core
LOB = 24                 # blocks per core in sub-shard A
LOA = LOB * BLK          # 3072 rows/core in sub-shard A
HIB = NPC - LOA          # 3200 rows/core in sub-shard B


# ---------------------------------------------------------------- host prep

def preprocess(edge_index):
    src = np.asarray(edge_index[0], dtype=np.int64)
    dst = np.asarray(edge_index[1], dtype=np.int64)

    # degree includes the self-loops even though they are not in the edge
    # stream (they are applied on-device via an identity matmul)
    deg = (np.bincount(dst, minlength=N) + 1).astype(np.float64)
    dinv = np.where(deg > 0, 1.0 / np.sqrt(deg), 0.0)
    sqrtdeg = np.where(deg > 0, np.sqrt(deg), 0.0)

    # snake-deal nodes (sorted by degree desc) into 392 blocks
    nblocks = NC * BPC
    order = np.argsort(-deg, kind="stable")
    perm = np.full(NPAD, -1, dtype=np.int64)   # new_id -> old_id ... actually old->new below
    node_new = np.empty(N, dtype=np.int64)
    counts = np.zeros(nblocks, dtype=np.int64)
    bi = 0
    direction = 1
    for i, nd in enumerate(order):
        # snake over blocks
        b = bi if direction == 1 else nblocks - 1 - bi
        node_new[nd] = b * BLK + counts[b]
        counts[b] += 1
        bi += 1
        if bi == nblocks:
            bi = 0
            direction = -direction
    assert counts.max() <= BLK

    # Edge halves (A/B sub-shard of the SRC) are fixed by which sub-shard a
    # node sits in; repacking nodes WITHIN a (core, sub-shard) never flips
    # any edge's half. Two host-only refinements under that invariant:
    # 1) 2D repack: redistribute nodes among their sub-shard's blocks to
    #    balance per-block (lo, hi) in-edge counts.
    # 2) slot alignment: within each sub-shard, order each core's blocks by
    #    lo-count so slot b holds similarly-sized groups on every core (the
    #    shared tile schedule takes max-over-cores per slot).
    s_p0 = node_new[src]
    d_p0 = node_new[dst]
    hf0 = ((s_p0 % NPC) >= LOA).astype(np.int64)
    node_lo = np.bincount(d_p0[hf0 == 0], minlength=NPAD).astype(np.int64)
    node_hi = np.bincount(d_p0[hf0 == 1], minlength=NPAD).astype(np.int64)

    new_pos = np.empty(NPAD, dtype=np.int64)
    for c in range(NC):
        for sh, (b0, nb) in enumerate(((0, LOB), (LOB, BPC - LOB))):
            rows = np.arange(c * NPC + b0 * BLK, c * NPC + (b0 + nb) * BLK)
            lo, hi = node_lo[rows], node_hi[rows]
            order = np.argsort(-(lo + hi), kind="stable")
            losum = np.zeros(nb); hisum = np.zeros(nb)
            nfill = np.zeros(nb, dtype=np.int64)
            assign = np.empty(len(rows), dtype=np.int64)
            for i in order:
                cost = (losum + lo[i]) ** 2 + (hisum + hi[i]) ** 2
                cost[nfill >= BLK] = np.inf
                j = int(np.argmin(cost))
                assign[i] = j
                losum[j] += lo[i]; hisum[j] += hi[i]; nfill[j] += 1
            # order blocks within the sub-shard by lo-count desc (slot align)
            slot_of = np.empty(nb, dtype=np.int64)
            slot_of[np.argsort(-losum, kind="stable")] = np.arange(nb)
            fill2 = np.zeros(nb, dtype=np.int64)
            for i in range(len(rows)):
                j = slot_of[assign[i]]
                new_pos[rows[i]] = c * NPC + (b0 + j) * BLK + fill2[j]
                fill2[j] += 1
    node_new = new_pos[node_new]

    s_p = node_new[src]
    d_p = node_new[dst]

    # per (core, block, half) edge lists
    core = d_p // NPC
    blk = (d_p % NPC) // BLK
    dloc = d_p % BLK
    score = s_p // NPC
    w = s_p % NPC
    hf = (w >= LOA).astype(np.int64)
    idxh = np.where(hf == 0, score * LOA + w, score * HIB + (w - LOA))

    # group: count matrix
    cnt = np.zeros((NC, BPC, 2), dtype=np.int64)
    np.add.at(cnt, (core, blk, hf), 1)
    T = np.maximum(1, np.ceil(cnt.max(axis=0) / BLK).astype(np.int64))  # [BPC, 2]

    offs = np.zeros((BPC, 2), dtype=np.int64)   # tile offset of each (b, hf) group
    t = 0
    for b in range(BPC):
        for h in range(2):
            offs[b, h] = t
            t += T[b, h]
    TT = t                                       # total tiles per core

    # build per-core idx / dstloc arrays
    idx_all = np.full((NC, TT * BLK), -1, dtype=np.int16)
    dloc_all = np.full((NC, TT * BLK), -1.0, dtype=np.float32)
    key = (core * BPC + blk) * 2 + hf
    ordkey = np.lexsort((idxh, key))
    ks = key[ordkey]
    sc, sb, sh = core[ordkey], blk[ordkey], hf[ordkey]
    si, sd = idxh[ordkey], dloc[ordkey]
    ne = len(ks)
    starts = np.r_[0, np.flatnonzero(np.diff(ks)) + 1]
    glen = np.diff(np.r_[starts, ne])
    pos = np.arange(ne) - np.repeat(starts, glen)
    slot = offs[sb, sh] * BLK + pos
    idx_all[sc, slot] = si.astype(np.int16)
    dloc_all[sc, slot] = sd.astype(np.float32)

    sd_pad = np.zeros(NPAD, dtype=np.float32)
    di_pad = np.zeros(NPAD, dtype=np.float32)
    sd_pad[node_new] = sqrtdeg
    di_pad[node_new] = dinv

    return dict(node_new=node_new, T=T, offs=offs, TT=TT, cnt=cnt,
                idx_all=idx_all, dloc_all=dloc_all,
                sqrtdeg=sd_pad, dinv=di_pad)


def make_inmaps(pre, x, weights):
    """weights: dict of padded bf16 weight/bias arrays (shared across cores)."""
    node_new = pre["node_new"]
    TT = pre["TT"]
    bf = ml_dtypes.bfloat16
    in_maps = []
    xpad = np.zeros((NPAD, 128), dtype=np.float32)
    xpad[node_new] = np.asarray(x, dtype=np.float32)
    for c in range(NC):
        m = {}
        xT = xpad[c * NPC:(c + 1) * NPC].T.copy()          # [128, 6272]
        m["xT"] = np.ascontiguousarray(xT, dtype=np.float32)
        # idx layout: token t -> [t%16, t//16], replicated to 128 partitions
        idx = pre["idx_all"][c]
        m["idxs"] = np.tile(idx.reshape(TT * 8, 16).T, (8, 1)).copy()
        dl = pre["dloc_all"][c].reshape(TT, BLK).T          # [128, TT]
        m["dstloc"] = np.ascontiguousarray(dl, dtype=bf)
        sl = slice(c * NPC, (c + 1) * NPC)
        gc = pre["cnt"][c].T.reshape(1, 2 * BPC)      # [1, 2*BPC], hf-major
        m["gcount"] = np.ascontiguousarray(gc, dtype=np.int32)
        m["sqrtdeg_row"] = pre["sqrtdeg"][sl][None, :].astype(bf)
        m["dinv_col"] = pre["dinv"][sl].reshape(BPC, BLK).T.astype(np.float32).copy()
        m["dinv2_col"] = (pre["dinv"][sl] ** 2).reshape(BPC, BLK).T.astype(np.float32).copy()
        Tmax = int(pre["T"].max())
        R = np.tile(np.arange(BLK, dtype=np.float32), (128, Tmax)).astype(bf)
        m["Rbig"] = R
        m["ident"] = np.eye(128, dtype=np.float32).astype(bf)
        m.update(weights)
        in_maps.append(m)
    return in_maps


def pad_weights(eW1, eb1, eW2, eb2, eWf, ebf, dW1, db1, dW2, db2, dWf, dbf):
    bf = ml_dtypes.bfloat16
    w = {}
    w["eW1"] = np.asarray(eW1, np.float32).astype(bf)                       # [128,128]
    eW2p = np.zeros((128, 128), np.float32); eW2p[:, :64] = eW2
    w["eW2p"] = eW2p.astype(bf)
    eWfp = np.zeros((128, 64), np.float32); eWfp[:64] = eWf
    w["eWfp"] = eWfp.astype(bf)
    w["dW1"] = np.asarray(dW1, np.float32).astype(bf)                       # [64,256]
    w["dW2"] = np.asarray(dW2, np.float32).astype(bf)                       # [256,128]
    w["dWf"] = np.asarray(dWf, np.float32).astype(bf)                       # [128,1024]
    eb1r = np.asarray(eb1, np.float32)[None, :]
    w["eb1_row"] = eb1r.astype(bf)                                          # [1,128]
    eb2r = np.zeros((1, 128), np.float32); eb2r[0, :64] = eb2
    w["eb2p_row"] = eb2r.astype(bf)
    w["ebf_col"] = np.asarray(ebf, np.float32)[:, None].astype(bf)          # [64,1]
    w["db1_row"] = np.asarray(db1, np.float32)[None, :].astype(bf)          # [1,256]
    w["db2_row"] = np.asarray(db2, np.float32)[None, :].astype(bf)          # [1,128]
    w["dbf_row"] = np.asarray(dbf, np.float32)[None, :].astype(bf)          # [1,1024]
    return w


# ---------------------------------------------------------------- device program

def build_program(T, offs, TT):
    """T: [BPC,2] tiles per (block,half); offs: [BPC,2] tile offsets; TT total tiles."""
    Tmax = int(T.max())
    nc = bacc.Bacc(None, target_bir_lowering=False)

    # ---- I/O
    xT_d = nc.dram_tensor("xT", [128, NPC], F32, kind="ExternalInput")
    idx_d = nc.dram_tensor("idxs", [128, TT * 8], I16, kind="ExternalInput")
    dloc_d = nc.dram_tensor("dstloc", [128, TT], BF16, kind="ExternalInput")
    gcount_d = nc.dram_tensor("gcount", [1, 2 * BPC], mybir.dt.int32, kind="ExternalInput")
    sqd_d = nc.dram_tensor("sqrtdeg_row", [1, NPC], BF16, kind="ExternalInput")
    dinv_d = nc.dram_tensor("dinv_col", [128, BPC], F32, kind="ExternalInput")
    dinv2_d = nc.dram_tensor("dinv2_col", [128, BPC], F32, kind="ExternalInput")
    R_d = nc.dram_tensor("Rbig", [128, Tmax * 128], BF16, kind="ExternalInput")
    id_d = nc.dram_tensor("ident", [128, 128], BF16, kind="ExternalInput")
    wnames = {"eW1": [128, 128], "eW2p": [128, 128], "eWfp": [128, 64],
              "dW1": [64, 256], "dW2": [256, 128], "dWf": [128, 1024],
              "eb1_row": [1, 128], "eb2p_row": [1, 128], "ebf_col": [64, 1],
              "db1_row": [1, 256], "db2_row": [1, 128], "dbf_row": [1, 1024]}
    w_d = {k: nc.dram_tensor(k, shp, BF16, kind="ExternalInput")
           for k, shp in wnames.items()}
    out_d = nc.dram_tensor("xhat", [2, NPC, 512], F32, kind="ExternalOutput")

    with tile.TileContext(nc) as tc:
        with tc.tile_pool(name="const", bufs=1) as cpool, \
             tc.tile_pool(name="acts", bufs=1) as apool, \
             tc.tile_pool(name="dram", bufs=1, space="DRAM") as dram:
            nc.gpsimd.load_library(mlp)

            # ---- persistent SBUF state
            w_sb = {}
            for k, shp in wnames.items():
                if shp[0] > 128:
                    continue
                t = cpool.tile(shp, BF16, name=f"w_{k}")
                nc.sync.dma_start(t[:], w_d[k][:])
                w_sb[k] = t
            dW2a = cpool.tile([128, 128], BF16, name="w_dW2a")
            nc.sync.dma_start(dW2a[:], w_d["dW2"][0:128, :])
            dW2b = cpool.tile([128, 128], BF16, name="w_dW2b")
            nc.sync.dma_start(dW2b[:], w_d["dW2"][128:256, :])
            idx_sb = cpool.tile([128, TT * 8], I16, name="idx_sb")
            nc.sync.dma_start(idx_sb[:], idx_d[:])
            dloc_sb = cpool.tile([128, TT], BF16, name="dloc_sb")
            nc.sync.dma_start(dloc_sb[:], dloc_d[:])
            gcount_sb = cpool.tile([1, 2 * BPC], mybir.dt.int32, name="gcount_sb")
            nc.sync.dma_start(gcount_sb[:], gcount_d[:])
            sqd_sb = cpool.tile([1, NPC], BF16, name="sqd_sb")
            nc.sync.dma_start(sqd_sb[:], sqd_d[:])
            dinv_sb = cpool.tile([128, BPC], F32, name="dinv_sb")
            nc.sync.dma_start(dinv_sb[:], dinv_d[:])
            dinv2_sb = cpool.tile([128, BPC], F32, name="dinv2_sb")
            nc.sync.dma_start(dinv2_sb[:], dinv2_d[:])
            R_sb = cpool.tile([128, Tmax * 128], BF16, name="R_sb")
            nc.sync.dma_start(R_sb[:], R_d[:])
            id_sb = cpool.tile([128, 128], BF16, name="id_sb")
            nc.sync.dma_start(id_sb[:], id_d[:])

            xT_sb = apool.tile([128, NPC], BF16, name="xT_sb")
            with tc.tile_pool(name="xload", bufs=1) as xp:
                xT_f32 = xp.tile([128, NPC], F32, name="xT_f32")
                nc.sync.dma_start(xT_f32[:], xT_d[:])
                nc.vector.tensor_copy(xT_sb[:], xT_f32[:])

            R3 = R_sb[:].rearrange("p (t d) -> p t d", d=128)

            uT = {}  # feature-major activation arrays per stage
            uT["u2"] = apool.tile([128, NPC], BF16, name="u2T")
            uT["w2"] = apool.tile([128, NPC], BF16, name="w2T")
            uT["v3"] = apool.tile([64, NPC], BF16, name="v3T")
            uT["u4a"] = apool.tile([128, NPC], BF16, name="u4aT")
            uT["u4b"] = apool.tile([128, NPC], BF16, name="u4bT")
            uT["f"] = apool.tile([128, NPC], BF16, name="fT")

            # ebf @ dW1 row (for L3 transform bias)
            ebfdW1 = cpool.tile([1, 256], BF16, name="ebfdW1")

            def nt_slice(ap, nt, width=128):
                return ap[:, nt * width:(nt + 1) * width]

            # ---------------- transform: h' = (rhsT-major u) @ W, node-major bf16 out
            def transform(layer, rhs_tiles, Ws, h_shard, scale_col, kk_parts):
                """rhs_tiles: list of (tile_ap, cin) feature-major [cin, NPC];
                Ws: list of (lhsT_ap per cinb) aligned with rhs_tiles; output ch
                = sum of W free widths per chb layout given by Ws2: here Ws is
                list over cinb of W slices [cin, ch]. h_shard: DRAM [NPC, ch]."""
                ch = Ws[0].shape[-1]
                nchb = (ch + 127) // 128
                with tc.tile_pool(name=f"t{layer}_ps", bufs=3, space="PSUM") as pps, \
                     tc.tile_pool(name=f"t{layer}_sb", bufs=4) as psb:
                    for nt in range(NT):
                        hn = psb.tile([128, ch], BF16, tag="hn")
                        for chb in range(nchb):
                            cw = min(128, ch - chb * 128)
                            ph = pps.tile([128, cw], F32, tag="ph")
                            for ki, (rt, cin) in enumerate(rhs_tiles):
                                nc.tensor.matmul(
                                    ph[:], Ws[ki][:, chb * 128:chb * 128 + cw],
                                    rt[:, nt * 128:(nt + 1) * 128],
                                    start=(ki == 0), stop=(ki == len(rhs_tiles) - 1))
                            hT = psb.tile([128, cw], BF16, tag="hT")
                            nc.scalar.activation(hT[:], ph[:],
                                                 mybir.ActivationFunctionType.Copy)
                            pn = pps.tile([128, cw], BF16, tag="pn")
                            nc.tensor.transpose(pn[:], hT[:], id_sb[:])
                            nc.scalar.activation(
                                hn[:, chb * 128:chb * 128 + cw], pn[:],
                                mybir.ActivationFunctionType.Copy,
                                scale=scale_col[:, nt:nt + 1])
                        if nt < LOB:
                            nc.sync.dma_start(h_shard[0][nt * 128:(nt + 1) * 128, :], hn[:])
                        else:
                            nc.sync.dma_start(h_shard[1][(nt - LOB) * 128:(nt - LOB + 1) * 128, :], hn[:])

            # ---------------- transform with extra rank-1 bias (L3: + ebfdW1 x sqrtdeg)
            def transform_bias(rhs_t, W, biasrow, h_shard, scale_col):
                cin, ch = W.shape
                nchb = ch // 128
                with tc.tile_pool(name="t3_ps", bufs=3, space="PSUM") as pps, \
                     tc.tile_pool(name="t3_sb", bufs=4) as psb:
                    for nt in range(NT):
                        hn = psb.tile([128, ch], BF16, tag="hn3")
                        for chb in range(nchb):
                            ph = pps.tile([128, 128], F32, tag="ph3")
                            nc.tensor.matmul(ph[:], W[:, chb * 128:(chb + 1) * 128],
                                             rhs_t[:, nt * 128:(nt + 1) * 128],
                                             start=True, stop=False)
                            nc.tensor.matmul(ph[:],
                                             biasrow[0:1, chb * 128:(chb + 1) * 128],
                                             sqd_sb[0:1, nt * 128:(nt + 1) * 128],
                                             start=False, stop=True)
                            hT = psb.tile([128, 128], BF16, tag="hT3")
                            nc.scalar.activation(hT[:], ph[:],
                                                 mybir.ActivationFunctionType.Copy)
                            pn = pps.tile([128, 128], BF16, tag="pn3")
                            nc.tensor.transpose(pn[:], hT[:], id_sb[:])
                            nc.scalar.activation(
                                hn[:, chb * 128:(chb + 1) * 128], pn[:],
                                mybir.ActivationFunctionType.Copy,
                                scale=scale_col[:, nt:nt + 1])
                        if nt < LOB:
                            nc.sync.dma_start(h_shard[0][nt * 128:(nt + 1) * 128, :], hn[:])
                        else:
                            nc.sync.dma_start(h_shard[1][(nt - LOB) * 128:(nt - LOB + 1) * 128, :], hn[:])

            # ---------------- SpMM: aggregate gathered msgs, feature-major out
            # pass A (sub-shard A sources) incl bias + self-loop identity mm,
            # then pass B accumulating on top; final add+relu merge on DVE.
            def spmm(layer, bufA, bufB, sA, sB, ch, bias_row, out_tiles, relu):
                nchb = ch // 128
                with tc.tile_pool(name=f"s{layer}_ps", bufs=3, space="PSUM") as pps, \
                     tc.tile_pool(name=f"s{layer}_m", bufs=4) as pm, \
                     tc.tile_pool(name=f"s{layer}_s", bufs=4) as psl, \
                     tc.tile_pool(name=f"s{layer}_h", bufs=3) as ph:
                    for hf in range(2):
                        buf = bufA if hf == 0 else bufB
                        for b in range(BPC):
                            Tb = int(T[b, hf]); off = int(offs[b, hf])
                            pbs = []
                            for chb in range(nchb):
                                pb = pps.tile([128, 128], F32, tag=f"pb{chb}")
                                pbs.append(pb)
                            if hf == 0:
                                hblk = ph.tile([128, ch], BF16, tag="hblk")
                                if b < LOB:
                                    nc.sync.dma_start(hblk[:], sA[b * 128:(b + 1) * 128, :])
                                else:
                                    nc.sync.dma_start(hblk[:], sB[(b - LOB) * 128:(b - LOB + 1) * 128, :])
                                for chb in range(nchb):
                                    nc.tensor.matmul(
                                        pbs[chb][:], bias_row[0:1, chb * 128:(chb + 1) * 128],
                                        sqd_sb[0:1, b * 128:(b + 1) * 128],
                                        start=True, stop=False)
                                    nc.tensor.matmul(
                                        pbs[chb][:], hblk[:, chb * 128:(chb + 1) * 128],
                                        id_sb[:], start=False, stop=False)
                            msg = pm.tile([128, Tmax, ch], BF16, tag="msg")
                            if b < 3:
                                nc.vector.memset(msg[:], 0.0)
                            creg = nc.gpsimd.alloc_register()
                            nc.gpsimd.load(creg, gcount_sb[0:1, hf * BPC + b:hf * BPC + b + 1])
                            nc.gpsimd.dma_gather(
                                msg[:, :Tb, :], buf[:],
                                idx_sb[:, off * 8:(off + Tb) * 8],
                                Tb * 128, creg, ch, single_packet=False)
                            S = psl.tile([128, Tmax, 128], BF16, tag="S")
                            d3 = dloc_sb[:, off:off + Tb].broadcast_to([128, Tb, 128])
                            nc.vector.tensor_tensor(S[:, :Tb, :], R3[:, :Tb, :], d3,
                                                    mybir.AluOpType.is_equal)
                            for t in range(Tb):
                                for chb in range(nchb):
                                    nc.tensor.matmul(
                                        pbs[chb][:],
                                        msg[:, t, chb * 128:(chb + 1) * 128],
                                        S[:, t, :],
                                        start=(hf == 1 and t == 0),
                                        stop=(t == Tb - 1))
                            for chb in range(nchb):
                                osl = out_tiles[chb][:, b * 128:(b + 1) * 128]
                                if hf == 0:
                                    nc.scalar.activation(
                                        osl, pbs[chb][:],
                                        mybir.ActivationFunctionType.Copy)
                                else:
                                    nc.vector.tensor_tensor(osl, osl, pbs[chb][:],
                                                            mybir.AluOpType.add)
                                    if relu:
                                        nc.vector.tensor_scalar(
                                            osl, osl, 0.0, None,
                                            mybir.AluOpType.max)

            # ================= network =================
            # L1 transform: h1' = x @ eW1, scale dinv
            h1sA = dram.tile([LOA, 128], BF16, name="h1_shardA")
            h1sB = dram.tile([HIB, 128], BF16, name="h1_shardB")
            h1fA = dram.tile([NC * LOA, 128], BF16, name="h1_fullA", addr_space="Shared")
            h1fB = dram.tile([NC * HIB, 128], BF16, name="h1_fullB", addr_space="Shared")
            transform(1, [(xT_sb, 128)], [w_sb["eW1"]], (h1sA[:], h1sB[:]), dinv_sb, 1)
            nc.gpsimd.collective_compute(
                "AllGather", mybir.AluOpType.bypass,
                replica_groups=[list(range(NC))],
                ins=[h1sA.opt()], outs=[h1fA.opt()])
            nc.gpsimd.collective_compute(
                "AllGather", mybir.AluOpType.bypass,
                replica_groups=[list(range(NC))],
                ins=[h1sB.opt()], outs=[h1fB.opt()])
            spmm(1, h1fA, h1fB, h1sA[:], h1sB[:], 128, w_sb["eb1_row"], [uT["u2"]], relu=True)

            # L2: h2' = u2 @ eW2p, scale dinv2
            h2sA = dram.tile([LOA, 128], BF16, name="h2_shardA")
            h2sB = dram.tile([HIB, 128], BF16, name="h2_shardB")
            h2fA = dram.tile([NC * LOA, 128], BF16, name="h2_fullA", addr_space="Shared")
            h2fB = dram.tile([NC * HIB, 128], BF16, name="h2_fullB", addr_space="Shared")
            transform(2, [(uT["u2"], 128)], [w_sb["eW2p"]], (h2sA[:], h2sB[:]), dinv2_sb, 1)
            nc.gpsimd.collective_compute(
                "AllGather", mybir.AluOpType.bypass,
                replica_groups=[list(range(NC))],
                ins=[h2sA.opt()], outs=[h2fA.opt()])
            nc.gpsimd.collective_compute(
                "AllGather", mybir.AluOpType.bypass,
                replica_groups=[list(range(NC))],
                ins=[h2sB.opt()], outs=[h2fB.opt()])
            spmm(2, h2fA, h2fB, h2sA[:], h2sB[:], 128, w_sb["eb2p_row"], [uT["w2"]], relu=False)

            # z-step: v3T = eWfp^T @ w2T  [64, NPC]
            with tc.tile_pool(name="z_ps", bufs=2, space="PSUM") as zps:
                pe = zps.tile([1, 256], F32, tag="pe")
                nc.tensor.matmul(pe[:], w_sb["ebf_col"][:], w_sb["dW1"][:],
                                 start=True, stop=True)
                nc.scalar.activation(ebfdW1[:], pe[:],
                                     mybir.ActivationFunctionType.Copy)
                for nt in range(NT):
                    pz = zps.tile([64, 128], F32, tag="pz")
                    nc.tensor.matmul(pz[:], w_sb["eWfp"][:],
                                     uT["w2"][:, nt * 128:(nt + 1) * 128],
                                     start=True, stop=True)
                    nc.scalar.activation(uT["v3"][:, nt * 128:(nt + 1) * 128], pz[:],
                                         mybir.ActivationFunctionType.Copy)

            # L3 transform: h3' = v3 @ dW1 + sqrtdeg x (ebf@dW1), scale dinv2
            h3sA = dram.tile([LOA, 256], BF16, name="h3_shardA")
            h3sB = dram.tile([HIB, 256], BF16, name="h3_shardB")
            h3fA = dram.tile([NC * LOA, 256], BF16, name="h3_fullA", addr_space="Shared")
            h3fB = dram.tile([NC * HIB, 256], BF16, name="h3_fullB", addr_space="Shared")
            transform_bias(uT["v3"], w_sb["dW1"], ebfdW1, (h3sA[:], h3sB[:]), dinv2_sb)
            nc.gpsimd.collective_compute(
                "AllGather", mybir.AluOpType.bypass,
                replica_groups=[list(range(NC))],
                ins=[h3sA.opt()], outs=[h3fA.opt()])
            nc.gpsimd.collective_compute(
                "AllGather", mybir.AluOpType.bypass,
                replica_groups=[list(range(NC))],
                ins=[h3sB.opt()], outs=[h3fB.opt()])
            spmm(3, h3fA, h3fB, h3sA[:], h3sB[:], 256, w_sb["db1_row"], [uT["u4a"], uT["u4b"]], relu=True)

            # L4: h4' = u4 @ dW2, scale dinv2
            h4sA = dram.tile([LOA, 128], BF16, name="h4_shardA")
            h4sB = dram.tile([HIB, 128], BF16, name="h4_shardB")
            h4fA = dram.tile([NC * LOA, 128], BF16, name="h4_fullA", addr_space="Shared")
            h4fB = dram.tile([NC * HIB, 128], BF16, name="h4_fullB", addr_space="Shared")
            transform(4, [(uT["u4a"], 128), (uT["u4b"], 128)],
                      [dW2a, dW2b],
                      (h4sA[:], h4sB[:]), dinv2_sb, 2)
            nc.gpsimd.collective_compute(
                "AllGather", mybir.AluOpType.bypass,
                replica_groups=[list(range(NC))],
                ins=[h4sA.opt()], outs=[h4fA.opt()])
            nc.gpsimd.collective_compute(
                "AllGather", mybir.AluOpType.bypass,
                replica_groups=[list(range(NC))],
                ins=[h4sB.opt()], outs=[h4fB.opt()])
            spmm(4, h4fA, h4fB, h4sA[:], h4sB[:], 128, w_sb["db2_row"], [uT["f"]], relu=False)

            # final: xhat = (dinv * (f + sqrtdeg x db... )) @ dWf + dbf
            with tc.tile_pool(name="f_ps", bufs=2, space="PSUM") as fps, \
                 tc.tile_pool(name="f_sb", bufs=3) as fsb:
                for nt in range(NT):
                    for cb in range(2):
                        pf = fps.tile([128, 512], F32, tag="pf")
                        nc.tensor.matmul(pf[:],
                                         uT["f"][:, nt * 128:(nt + 1) * 128],
                                         w_sb["dWf"][:, cb * 512:(cb + 1) * 512],
                                         start=True, stop=False)
                        nc.tensor.matmul(pf[:],
                                         sqd_sb[0:1, nt * 128:(nt + 1) * 128],
                                         w_sb["dbf_row"][0:1, cb * 512:(cb + 1) * 512],
                                         start=False, stop=True)
                        ob = fsb.tile([128, 512], F32, tag="ob")
                        nc.scalar.activation(ob[:], pf[:],
                                             mybir.ActivationFunctionType.Copy,
                                             scale=dinv_sb[:, nt:nt + 1])
                        nc.sync.dma_start(out_d[cb, nt * 128:(nt + 1) * 128, :], ob[:])

    nc.finalize()
    return nc


# ---------------------------------------------------------------- entry point

def kernel(x, edge_index, eW1, eb1, eW2, eb2, eWf, ebf,
           dW1, db1, dW2, db2, dWf, dbf):
    x = np.asarray(x, dtype=np.float32)
    edge_index = np.asarray(edge_index)

    pre = preprocess(edge_index)
    w = pad_weights(eW1, eb1, eW2, eb2, eWf, ebf, dW1, db1, dW2, db2, dWf, dbf)
    in_maps = make_inmaps(pre, x, w)
    nc = build_program(pre["T"], pre["offs"], pre["TT"])

    trace = os.environ.get("GCAE_TRACE", "0") == "1"
    if trace:
        trace = _install_profile_hook()
    res = None
    last_err = None
    for attempt in range(3):
        try:
            res = run_bass_kernel_spmd(nc, in_maps, core_ids=list(range(NC)),
                                       trace=trace and attempt == 0)
            break
        except Exception as e:  # transient device wedge: retry, drop tracing
            last_err = e
    if res is None:
        raise last_err
    if trace and res.exec_time_ns:
        print(f"HW exec time: {res.exec_time_ns} ns")

    xhat_pad = np.empty((NPAD, 1024), dtype=np.float32)
    for c in range(NC):
        o = np.asarray(res.results[c]["xhat"])
        xhat_pad[c * NPC:(c + 1) * NPC, 0:512] = o[0]
        xhat_pad[c * NPC:(c + 1) * NPC, 512:1024] = o[1]
    return xhat_pad[pre["node_new"]]



# revision 8
# speedup vs baseline: 3.1473x; 3.1473x over previous
"""Trainium2 Bass kernel for a 6-layer GCN autoencoder (50k nodes, 800k edges).

Self-contained: host-side graph preprocessing (node permutation/packing,
edge binning, degree norms), an 8-core SPMD Bass/Tile program (feature-major
dataflow, one-hot scatter matmuls, dma_gather halo reads, AllGather
collectives), and output assembly.
"""
import sys
sys.path.insert(0, '/opt/trn_rl_repo')

import contextlib
import ctypes
import os
import types

import numpy as np
import ml_dtypes

import concourse.bacc as bacc
import concourse.bass as bass
import concourse.mybir as mybir
import concourse.tile as tile
from concourse.library_config import mlp
from concourse.vector_clock import ScopedClock
from concourse.bass_utils import run_bass_kernel_spmd


# ---- workaround: this walrus build rejects >2 sync waits on one instruction;
# spread Tile's tail-drain waits across single-wait SP NOPs.
def _patched_drain_and_barrier(self, tick_clock, wait_clock):
    nc = self.nc
    probe = nc.sync.nop()
    wait_clock.add_sem_waits(probe.ins, ScopedClock({None: tick_clock.global_clock}))
    si = probe.ins.sync_info
    waits = list(si.on_wait) if si is not None else []
    if si is not None:
        while si.on_wait:
            si.on_wait.pop()
    for w in waits:
        n = nc.sync.nop()
        n.ins.sync_info = mybir.SyncInfo(on_wait=[w], on_update=[])
    nc.sync.drain()
    nc.all_engine_barrier()
    assert self.sems is not None
    popped = nc._tile_sem_poison_stack.pop()
    assert popped is self._sem_poison
    nc.clear_and_free_semaphores(list(self.sems.allocated().values()))
    nc.all_engine_barrier()


tile.TileContext._drain_and_barrier = _patched_drain_and_barrier


# ---- optional NTFF profiling hook (GCAE_TRACE=1)
def _install_profile_hook():
    try:
        import antenv
    except ImportError:
        return False
    if getattr(antenv, "axon_hooks", None) is not None:
        return True
    so_path = "/opt/axon/libaxon_pjrt.so"
    if not os.path.exists(so_path):
        return False
    lib = ctypes.CDLL(so_path)
    if not hasattr(lib, "axon_start_nrt_profile"):
        return False
    lib.axon_start_nrt_profile.argtypes = [ctypes.POINTER(ctypes.c_int64), ctypes.c_size_t]
    lib.axon_start_nrt_profile.restype = ctypes.c_int64
    lib.axon_stop_nrt_profile.argtypes = [ctypes.c_char_p]
    lib.axon_stop_nrt_profile.restype = ctypes.c_int64

    @contextlib.contextmanager
    def _hook(output_dir, device_ids):
        import jax
        jax.devices()
        if device_ids:
            ids = (ctypes.c_int64 * len(device_ids))(*device_ids)
            rc = lib.axon_start_nrt_profile(ids, len(device_ids))
        else:
            rc = lib.axon_start_nrt_profile(None, 0)
        if rc != 0:
            raise RuntimeError(f"axon_start_nrt_profile rc={rc}")
        try:
            yield
        finally:
            n = lib.axon_stop_nrt_profile(str(output_dir).encode())
            if n < 0:
                raise RuntimeError(f"axon_stop_nrt_profile rc={n}")

    hooks = types.ModuleType("antenv.axon_hooks")
    _h = [_hook]
    hooks.set_axon_ntff_profile_hook = lambda h: _h.__setitem__(0, h)
    hooks.get_axon_ntff_profile_hook = lambda: _h[0]
    sys.modules["antenv.axon_hooks"] = hooks
    antenv.axon_hooks = hooks
    return True

F32 = mybir.dt.float32
BF16 = mybir.dt.bfloat16
I16 = mybir.dt.int16

N = 50000
NC = 8
BLK = 128
BPC = 49                 # blocks per core
NPC = BPC * BLK          # 6272 nodes per core
NPAD = NC * NPC          # 50176
HALF = NPAD // 2         # 25088
NT = BPC                 # node tiles per core
LOB = 24                 # blocks per core in sub-shard A
LOA = LOB * BLK          # 3072 rows/core in sub-shard A
HIB = NPC - LOA          # 3200 rows/core in sub-shard B


# ---------------------------------------------------------------- host prep

def preprocess(edge_index):
    src = np.asarray(edge_index[0], dtype=np.int64)
    dst = np.asarray(edge_index[1], dtype=np.int64)

    # degree includes the self-loops even though they are not in the edge
    # stream (they are applied on-device via an identity matmul)
    deg = (np.bincount(dst, minlength=N) + 1).astype(np.float64)
    dinv = np.where(deg > 0, 1.0 / np.sqrt(deg), 0.0)
    sqrtdeg = np.where(deg > 0, np.sqrt(deg), 0.0)

    # snake-deal nodes (sorted by degree desc) into 392 blocks
    nblocks = NC * BPC
    order = np.argsort(-deg, kind="stable")
    perm = np.full(NPAD, -1, dtype=np.int64)   # new_id -> old_id ... actually old->new below
    node_new = np.empty(N, dtype=np.int64)
    counts = np.zeros(nblocks, dtype=np.int64)
    bi = 0
    direction = 1
    for i, nd in enumerate(order):
        # snake over blocks
        b = bi if direction == 1 else nblocks - 1 - bi
        node_new[nd] = b * BLK + counts[b]
        counts[b] += 1
        bi += 1
        if bi == nblocks:
            bi = 0
            direction = -direction
    assert counts.max() <= BLK

    # Edge halves (A/B sub-shard of the SRC) are fixed by which sub-shard a
    # node sits in; repacking nodes WITHIN a (core, sub-shard) never flips
    # any edge's half. Two host-only refinements under that invariant:
    # 1) 2D repack: redistribute nodes among their sub-shard's blocks to
    #    balance per-block (lo, hi) in-edge counts.
    # 2) slot alignment: within each sub-shard, order each core's blocks by
    #    lo-count so slot b holds similarly-sized groups on every core (the
    #    shared tile schedule takes max-over-cores per slot).
    s_p0 = node_new[src]
    d_p0 = node_new[dst]
    hf0 = ((s_p0 % NPC) >= LOA).astype(np.int64)
    node_lo = np.bincount(d_p0[hf0 == 0], minlength=NPAD).astype(np.int64)
    node_hi = np.bincount(d_p0[hf0 == 1], minlength=NPAD).astype(np.int64)

    new_pos = np.empty(NPAD, dtype=np.int64)
    for c in range(NC):
        for sh, (b0, nb) in enumerate(((0, LOB), (LOB, BPC - LOB))):
            rows = np.arange(c * NPC + b0 * BLK, c * NPC + (b0 + nb) * BLK)
            lo, hi = node_lo[rows], node_hi[rows]
            order = np.argsort(-(lo + hi), kind="stable")
            losum = np.zeros(nb); hisum = np.zeros(nb)
            nfill = np.zeros(nb, dtype=np.int64)
            assign = np.empty(len(rows), dtype=np.int64)
            for i in order:
                cost = (losum + lo[i]) ** 2 + (hisum + hi[i]) ** 2
                cost[nfill >= BLK] = np.inf
                j = int(np.argmin(cost))
                assign[i] = j
                losum[j] += lo[i]; hisum[j] += hi[i]; nfill[j] += 1
            # order blocks within the sub-shard by lo-count desc (slot align)
            slot_of = np.empty(nb, dtype=np.int64)
            slot_of[np.argsort(-losum, kind="stable")] = np.arange(nb)
            fill2 = np.zeros(nb, dtype=np.int64)
            for i in range(len(rows)):
                j = slot_of[assign[i]]
                new_pos[rows[i]] = c * NPC + (b0 + j) * BLK + fill2[j]
                fill2[j] += 1
    node_new = new_pos[node_new]

    s_p = node_new[src]
    d_p = node_new[dst]

    # per (core, block, half) edge lists
    core = d_p // NPC
    blk = (d_p % NPC) // BLK
    dloc = d_p % BLK
    score = s_p // NPC
    w = s_p % NPC
    hf = (w >= LOA).astype(np.int64)
    idxh = np.where(hf == 0, score * LOA + w, score * HIB + (w - LOA))

    # group: count matrix
    cnt = np.zeros((NC, BPC, 2), dtype=np.int64)
    np.add.at(cnt, (core, blk, hf), 1)
    T = np.maximum(1, np.ceil(cnt.max(axis=0) / BLK).astype(np.int64))  # [BPC, 2]

    offs = np.zeros((BPC, 2), dtype=np.int64)   # tile offset of each (b, hf) group
    t = 0
    for b in range(BPC):
        for h in range(2):
            offs[b, h] = t
            t += T[b, h]
    TT = t                                       # total tiles per core

    # build per-core idx / dstloc arrays
    idx_all = np.full((NC, TT * BLK), -1, dtype=np.int16)
    dloc_all = np.full((NC, TT * BLK), -1.0, dtype=np.float32)
    key = (core * BPC + blk) * 2 + hf
    ordkey = np.lexsort((idxh, key))
    ks = key[ordkey]
    sc, sb, sh = core[ordkey], blk[ordkey], hf[ordkey]
    si, sd = idxh[ordkey], dloc[ordkey]
    ne = len(ks)
    starts = np.r_[0, np.flatnonzero(np.diff(ks)) + 1]
    glen = np.diff(np.r_[starts, ne])
    pos = np.arange(ne) - np.repeat(starts, glen)
    slot = offs[sb, sh] * BLK + pos
    idx_all[sc, slot] = si.astype(np.int16)
    dloc_all[sc, slot] = sd.astype(np.float32)

    sd_pad = np.zeros(NPAD, dtype=np.float32)
    di_pad = np.zeros(NPAD, dtype=np.float32)
    sd_pad[node_new] = sqrtdeg
    di_pad[node_new] = dinv

    return dict(node_new=node_new, T=T, offs=offs, TT=TT, cnt=cnt,
                idx_all=idx_all, dloc_all=dloc_all,
                sqrtdeg=sd_pad, dinv=di_pad)


def make_inmaps(pre, x, weights):
    """weights: dict of padded bf16 weight/bias arrays (shared across cores)."""
    node_new = pre["node_new"]
    TT = pre["TT"]
    T2max = int((pre["T"][:, 0] + pre["T"][:, 1]).max())
    bf = ml_dtypes.bfloat16
    in_maps = []
    xpad = np.zeros((NPAD, 128), dtype=np.float32)
    xpad[node_new] = np.asarray(x, dtype=np.float32)
    for c in range(NC):
        m = {}
        xT = xpad[c * NPC:(c + 1) * NPC].T.copy()          # [128, 6272]
        m["xT"] = np.ascontiguousarray(xT, dtype=np.float32)
        # idx layout: token t -> [t%16, t//16], replicated to 128 partitions
        idx = pre["idx_all"][c]
        m["idxs"] = np.tile(idx.reshape(TT * 8, 16).T, (8, 1)).copy()
        dl = pre["dloc_all"][c].reshape(TT, BLK).T          # [128, TT]
        m["dstloc"] = np.ascontiguousarray(dl, dtype=bf)
        sl = slice(c * NPC, (c + 1) * NPC)
        gc = pre["cnt"][c].T.reshape(1, 2 * BPC)      # [1, 2*BPC], hf-major
        m["gcount"] = np.ascontiguousarray(gc, dtype=np.int32)
        m["sqrtdeg_row"] = pre["sqrtdeg"][sl][None, :].astype(bf)
        m["dinv_col"] = pre["dinv"][sl].reshape(BPC, BLK).T.astype(np.float32).copy()
        m["dinv2_col"] = (pre["dinv"][sl] ** 2).reshape(BPC, BLK).T.astype(np.float32).copy()
        R = np.tile(np.arange(BLK, dtype=np.float32), (128, T2max)).astype(bf)
        m["Rbig"] = R
        m["ident"] = np.eye(128, dtype=np.float32).astype(bf)
        m.update(weights)
        in_maps.append(m)
    return in_maps


def pad_weights(eW1, eb1, eW2, eb2, eWf, ebf, dW1, db1, dW2, db2, dWf, dbf):
    bf = ml_dtypes.bfloat16
    w = {}
    w["eW1"] = np.asarray(eW1, np.float32).astype(bf)                       # [128,128]
    eW2p = np.zeros((128, 128), np.float32); eW2p[:, :64] = eW2
    w["eW2p"] = eW2p.astype(bf)
    eWfp = np.zeros((128, 64), np.float32); eWfp[:64] = eWf
    w["eWfp"] = eWfp.astype(bf)
    w["dW1"] = np.asarray(dW1, np.float32).astype(bf)                       # [64,256]
    w["dW2"] = np.asarray(dW2, np.float32).astype(bf)                       # [256,128]
    w["dWf"] = np.asarray(dWf, np.float32).astype(bf)                       # [128,1024]
    eb1r = np.asarray(eb1, np.float32)[None, :]
    w["eb1_row"] = eb1r.astype(bf)                                          # [1,128]
    eb2r = np.zeros((1, 128), np.float32); eb2r[0, :64] = eb2
    w["eb2p_row"] = eb2r.astype(bf)
    w["ebf_col"] = np.asarray(ebf, np.float32)[:, None].astype(bf)          # [64,1]
    w["db1_row"] = np.asarray(db1, np.float32)[None, :].astype(bf)          # [1,256]
    w["db2_row"] = np.asarray(db2, np.float32)[None, :].astype(bf)          # [1,128]
    w["dbf_row"] = np.asarray(dbf, np.float32)[None, :].astype(bf)          # [1,1024]
    return w


# ---------------------------------------------------------------- device program

def build_program(T, offs, TT):
    """T: [BPC,2] tiles per (block,half); offs: [BPC,2] tile offsets; TT total tiles."""
    Tmax = int(T.max())
    T2max = int((T[:, 0] + T[:, 1]).max())
    nc = bacc.Bacc(None, target_bir_lowering=False, num_swdge_queues=4)

    # ---- I/O
    xT_d = nc.dram_tensor("xT", [128, NPC], F32, kind="ExternalInput")
    idx_d = nc.dram_tensor("idxs", [128, TT * 8], I16, kind="ExternalInput")
    dloc_d = nc.dram_tensor("dstloc", [128, TT], BF16, kind="ExternalInput")
    gcount_d = nc.dram_tensor("gcount", [1, 2 * BPC], mybir.dt.int32, kind="ExternalInput")
    sqd_d = nc.dram_tensor("sqrtdeg_row", [1, NPC], BF16, kind="ExternalInput")
    dinv_d = nc.dram_tensor("dinv_col", [128, BPC], F32, kind="ExternalInput")
    dinv2_d = nc.dram_tensor("dinv2_col", [128, BPC], F32, kind="ExternalInput")
    R_d = nc.dram_tensor("Rbig", [128, T2max * 128], BF16, kind="ExternalInput")
    id_d = nc.dram_tensor("ident", [128, 128], BF16, kind="ExternalInput")
    wnames = {"eW1": [128, 128], "eW2p": [128, 128], "eWfp": [128, 64],
              "dW1": [64, 256], "dW2": [256, 128], "dWf": [128, 1024],
              "eb1_row": [1, 128], "eb2p_row": [1, 128], "ebf_col": [64, 1],
              "db1_row": [1, 256], "db2_row": [1, 128], "dbf_row": [1, 1024]}
    w_d = {k: nc.dram_tensor(k, shp, BF16, kind="ExternalInput")
           for k, shp in wnames.items()}
    out_d = nc.dram_tensor("xhat", [2, NPC, 512], F32, kind="ExternalOutput")

    with tile.TileContext(nc) as tc:
        with tc.tile_pool(name="const", bufs=1) as cpool, \
             tc.tile_pool(name="acts", bufs=1) as apool, \
             tc.tile_pool(name="dram", bufs=1, space="DRAM") as dram:
            nc.gpsimd.load_library(mlp)

            # ---- persistent SBUF state
            w_sb = {}
            for k, shp in wnames.items():
                if shp[0] > 128:
                    continue
                t = cpool.tile(shp, BF16, name=f"w_{k}")
                nc.sync.dma_start(t[:], w_d[k][:])
                w_sb[k] = t
            dW2a = cpool.tile([128, 128], BF16, name="w_dW2a")
            nc.sync.dma_start(dW2a[:], w_d["dW2"][0:128, :])
            dW2b = cpool.tile([128, 128], BF16, name="w_dW2b")
            nc.sync.dma_start(dW2b[:], w_d["dW2"][128:256, :])
            idx_sb = cpool.tile([128, TT * 8], I16, name="idx_sb")
            nc.sync.dma_start(idx_sb[:], idx_d[:])
            dloc_sb = cpool.tile([128, TT], BF16, name="dloc_sb")
            nc.sync.dma_start(dloc_sb[:], dloc_d[:])
            gcount_sb = cpool.tile([1, 2 * BPC], mybir.dt.int32, name="gcount_sb")
            nc.sync.dma_start(gcount_sb[:], gcount_d[:])
            sqd_sb = cpool.tile([1, NPC], BF16, name="sqd_sb")
            nc.sync.dma_start(sqd_sb[:], sqd_d[:])
            dinv_sb = cpool.tile([128, BPC], F32, name="dinv_sb")
            nc.sync.dma_start(dinv_sb[:], dinv_d[:])
            dinv2_sb = cpool.tile([128, BPC], F32, name="dinv2_sb")
            nc.sync.dma_start(dinv2_sb[:], dinv2_d[:])
            R_sb = cpool.tile([128, T2max * 128], BF16, name="R_sb")
            nc.sync.dma_start(R_sb[:], R_d[:])
            id_sb = cpool.tile([128, 128], BF16, name="id_sb")
            nc.sync.dma_start(id_sb[:], id_d[:])

            xT_sb = apool.tile([128, NPC], BF16, name="xT_sb")
            with tc.tile_pool(name="xload", bufs=1) as xp:
                xT_f32 = xp.tile([128, NPC], F32, name="xT_f32")
                nc.sync.dma_start(xT_f32[:], xT_d[:])
                nc.vector.tensor_copy(xT_sb[:], xT_f32[:])

            R3 = R_sb[:].rearrange("p (t d) -> p t d", d=128)

            uT = {}  # feature-major activation arrays per stage
            uT["u2"] = apool.tile([128, NPC], BF16, name="u2T")
            uT["w2"] = apool.tile([128, NPC], BF16, name="w2T")
            uT["v3"] = apool.tile([64, NPC], BF16, name="v3T")
            uT["u4a"] = apool.tile([128, NPC], BF16, name="u4aT")
            uT["u4b"] = apool.tile([128, NPC], BF16, name="u4bT")
            uT["f"] = apool.tile([128, NPC], BF16, name="fT")

            # ebf @ dW1 row (for L3 transform bias)
            ebfdW1 = cpool.tile([1, 256], BF16, name="ebfdW1")

            def nt_slice(ap, nt, width=128):
                return ap[:, nt * width:(nt + 1) * width]

            # ---------------- transform: h' = (rhsT-major u) @ W, node-major bf16 out
            def transform(layer, rhs_tiles, Ws, h_shard, scale_col, kk_parts):
                """rhs_tiles: list of (tile_ap, cin) feature-major [cin, NPC];
                Ws: list of (lhsT_ap per cinb) aligned with rhs_tiles; output ch
                = sum of W free widths per chb layout given by Ws2: here Ws is
                list over cinb of W slices [cin, ch]. h_shard: DRAM [NPC, ch]."""
                ch = Ws[0].shape[-1]
                nchb = (ch + 127) // 128
                with tc.tile_pool(name=f"t{layer}_ps", bufs=3, space="PSUM") as pps, \
                     tc.tile_pool(name=f"t{layer}_sb", bufs=4) as psb:
                    for nt in range(NT):
                        hn = psb.tile([128, ch], BF16, tag="hn")
                        for chb in range(nchb):
                            cw = min(128, ch - chb * 128)
                            ph = pps.tile([128, cw], F32, tag="ph")
                            for ki, (rt, cin) in enumerate(rhs_tiles):
                                nc.tensor.matmul(
                                    ph[:], Ws[ki][:, chb * 128:chb * 128 + cw],
                                    rt[:, nt * 128:(nt + 1) * 128],
                                    start=(ki == 0), stop=(ki == len(rhs_tiles) - 1))
                            hT = psb.tile([128, cw], BF16, tag="hT")
                            nc.scalar.activation(hT[:], ph[:],
                                                 mybir.ActivationFunctionType.Copy)
                            pn = pps.tile([128, cw], BF16, tag="pn")
                            nc.tensor.transpose(pn[:], hT[:], id_sb[:])
                            nc.scalar.activation(
                                hn[:, chb * 128:chb * 128 + cw], pn[:],
                                mybir.ActivationFunctionType.Copy,
                                scale=scale_col[:, nt:nt + 1])
                        if nt < LOB:
                            nc.sync.dma_start(h_shard[0][nt * 128:(nt + 1) * 128, :], hn[:])
                        else:
                            nc.sync.dma_start(h_shard[1][(nt - LOB) * 128:(nt - LOB + 1) * 128, :], hn[:])

            # ---------------- transform with extra rank-1 bias (L3: + ebfdW1 x sqrtdeg)
            def transform_bias(rhs_t, W, biasrow, h_shard, scale_col):
                cin, ch = W.shape
                nchb = ch // 128
                with tc.tile_pool(name="t3_ps", bufs=3, space="PSUM") as pps, \
                     tc.tile_pool(name="t3_sb", bufs=4) as psb:
                    for nt in range(NT):
                        hn = psb.tile([128, ch], BF16, tag="hn3")
                        for chb in range(nchb):
                            ph = pps.tile([128, 128], F32, tag="ph3")
                            nc.tensor.matmul(ph[:], W[:, chb * 128:(chb + 1) * 128],
                                             rhs_t[:, nt * 128:(nt + 1) * 128],
                                             start=True, stop=False)
                            nc.tensor.matmul(ph[:],
                                             biasrow[0:1, chb * 128:(chb + 1) * 128],
                                             sqd_sb[0:1, nt * 128:(nt + 1) * 128],
                                             start=False, stop=True)
                            hT = psb.tile([128, 128], BF16, tag="hT3")
                            nc.scalar.activation(hT[:], ph[:],
                                                 mybir.ActivationFunctionType.Copy)
                            pn = pps.tile([128, 128], BF16, tag="pn3")
                            nc.tensor.transpose(pn[:], hT[:], id_sb[:])
                            nc.scalar.activation(
                                hn[:, chb * 128:(chb + 1) * 128], pn[:],
                                mybir.ActivationFunctionType.Copy,
                                scale=scale_col[:, nt:nt + 1])
                        if nt < LOB:
                            nc.sync.dma_start(h_shard[0][nt * 128:(nt + 1) * 128, :], hn[:])
                        else:
                            nc.sync.dma_start(h_shard[1][(nt - LOB) * 128:(nt - LOB + 1) * 128, :], hn[:])

            # ---------------- SpMM: aggregate gathered msgs, feature-major out
            # pass A (sub-shard A sources) incl bias + self-loop identity mm,
            # writes bf16 partial to out_tiles; pass B re-injects the partial
            # into its PSUM chain via an identity matmul so the final copy-out
            # (with fused relu) happens on ACT — no DVE merge ops. dma_gathers
            # are striped over the 4 SWDGE queues so all four Q7 core pairs
            # generate descriptors concurrently.
            qstate = [0]

            def spmm(layer, bufA, bufB, sA, sB, ch, bias_row, out_tiles, relu):
                nchb = ch // 128
                with tc.tile_pool(name=f"s{layer}_ps", bufs=(4 if nchb == 1 else 3),
                                  space="PSUM") as pps, \
                     tc.tile_pool(name=f"s{layer}_m", bufs=8) as pm, \
                     tc.tile_pool(name=f"s{layer}_s", bufs=4) as psl, \
                     tc.tile_pool(name=f"s{layer}_h", bufs=3) as ph:
                    for hf in range(2):
                        buf = bufA if hf == 0 else bufB
                        for b in range(BPC):
                            Tb = int(T[b, hf]); off = int(offs[b, hf])
                            pbs = []
                            for chb in range(nchb):
                                pb = pps.tile([128, 128], F32, tag=f"pb{chb}")
                                pbs.append(pb)
                            if hf == 0:
                                hblk = ph.tile([128, ch], BF16, tag="hblk")
                                if b < LOB:
                                    nc.sync.dma_start(hblk[:], sA[b * 128:(b + 1) * 128, :])
                                else:
                                    nc.sync.dma_start(hblk[:], sB[(b - LOB) * 128:(b - LOB + 1) * 128, :])
                                for chb in range(nchb):
                                    nc.tensor.matmul(
                                        pbs[chb][:], bias_row[0:1, chb * 128:(chb + 1) * 128],
                                        sqd_sb[0:1, b * 128:(b + 1) * 128],
                                        start=True, stop=False)
                                    nc.tensor.matmul(
                                        pbs[chb][:], hblk[:, chb * 128:(chb + 1) * 128],
                                        id_sb[:], start=False, stop=False)
                            else:
                                # re-inject pass-A partial: pbs = I^T @ osl
                                for chb in range(nchb):
                                    nc.tensor.matmul(
                                        pbs[chb][:], id_sb[:],
                                        out_tiles[chb][:, b * 128:(b + 1) * 128],
                                        start=True, stop=False)
                            msg = pm.tile([128, Tmax, ch], BF16, tag="msg")
                            if hf == 0 and b < 8:
                                nc.vector.memset(msg[:], 0.0)
                            creg = nc.gpsimd.alloc_register()
                            nc.gpsimd.load(creg, gcount_sb[0:1, hf * BPC + b:hf * BPC + b + 1])
                            nc.gpsimd.dma_gather(
                                msg[:, :Tb, :], buf[:],
                                idx_sb[:, off * 8:(off + Tb) * 8],
                                Tb * 128, creg, ch, single_packet=False,
                                queue_num=qstate[0])
                            qstate[0] = (qstate[0] + 1) % 4
                            S = psl.tile([128, Tmax, 128], BF16, tag="S")
                            d3 = dloc_sb[:, off:off + Tb].broadcast_to([128, Tb, 128])
                            nc.vector.tensor_tensor(S[:, :Tb, :], R3[:, :Tb, :], d3,
                                                    mybir.AluOpType.is_equal)
                            for t in range(Tb):
                                for chb in range(nchb):
                                    nc.tensor.matmul(
                                        pbs[chb][:],
                                        msg[:, t, chb * 128:(chb + 1) * 128],
                                        S[:, t, :],
                                        start=False,
                                        stop=(t == Tb - 1))
                            for chb in range(nchb):
                                osl = out_tiles[chb][:, b * 128:(b + 1) * 128]
                                nc.scalar.activation(
                                    osl, pbs[chb][:],
                                    mybir.ActivationFunctionType.Relu
                                    if (relu and hf == 1)
                                    else mybir.ActivationFunctionType.Copy)

            # ================= network =================
            # L1 transform: h1' = x @ eW1, scale dinv
            h1sA = dram.tile([LOA, 128], BF16, name="h1_shardA")
            h1sB = dram.tile([HIB, 128], BF16, name="h1_shardB")
            h1fA = dram.tile([NC * LOA, 128], BF16, name="h1_fullA", addr_space="Shared")
            h1fB = dram.tile([NC * HIB, 128], BF16, name="h1_fullB", addr_space="Shared")
            transform(1, [(xT_sb, 128)], [w_sb["eW1"]], (h1sA[:], h1sB[:]), dinv_sb, 1)
            nc.gpsimd.collective_compute(
                "AllGather", mybir.AluOpType.bypass,
                replica_groups=[list(range(NC))],
                ins=[h1sA.opt()], outs=[h1fA.opt()])
            nc.gpsimd.collective_compute(
                "AllGather", mybir.AluOpType.bypass,
                replica_groups=[list(range(NC))],
                ins=[h1sB.opt()], outs=[h1fB.opt()])
            spmm(1, h1fA, h1fB, h1sA[:], h1sB[:], 128, w_sb["eb1_row"], [uT["u2"]], relu=True)

            # L2: h2' = u2 @ eW2p, scale dinv2
            h2sA = dram.tile([LOA, 128], BF16, name="h2_shardA")
            h2sB = dram.tile([HIB, 128], BF16, name="h2_shardB")
            h2fA = dram.tile([NC * LOA, 128], BF16, name="h2_fullA", addr_space="Shared")
            h2fB = dram.tile([NC * HIB, 128], BF16, name="h2_fullB", addr_space="Shared")
            transform(2, [(uT["u2"], 128)], [w_sb["eW2p"]], (h2sA[:], h2sB[:]), dinv2_sb, 1)
            nc.gpsimd.collective_compute(
                "AllGather", mybir.AluOpType.bypass,
                replica_groups=[list(range(NC))],
                ins=[h2sA.opt()], outs=[h2fA.opt()])
            nc.gpsimd.collective_compute(
                "AllGather", mybir.AluOpType.bypass,
                replica_groups=[list(range(NC))],
                ins=[h2sB.opt()], outs=[h2fB.opt()])
            spmm(2, h2fA, h2fB, h2sA[:], h2sB[:], 128, w_sb["eb2p_row"], [uT["w2"]], relu=False)

            # z-step: v3T = eWfp^T @ w2T  [64, NPC]
            with tc.tile_pool(name="z_ps", bufs=2, space="PSUM") as zps:
                pe = zps.tile([1, 256], F32, tag="pe")
                nc.tensor.matmul(pe[:], w_sb["ebf_col"][:], w_sb["dW1"][:],
                                 start=True, stop=True)
                nc.scalar.activation(ebfdW1[:], pe[:],
                                     mybir.ActivationFunctionType.Copy)
                for nt in range(NT):
                    pz = zps.tile([64, 128], F32, tag="pz")
                    nc.tensor.matmul(pz[:], w_sb["eWfp"][:],
                                     uT["w2"][:, nt * 128:(nt + 1) * 128],
                                     start=True, stop=True)
                    nc.scalar.activation(uT["v3"][:, nt * 128:(nt + 1) * 128], pz[:],
                                         mybir.ActivationFunctionType.Copy)

            # L3 transform: h3' = v3 @ dW1 + sqrtdeg x (ebf@dW1), scale dinv2
            h3sA = dram.tile([LOA, 256], BF16, name="h3_shardA")
            h3sB = dram.tile([HIB, 256], BF16, name="h3_shardB")
            h3fA = dram.tile([NC * LOA, 256], BF16, name="h3_fullA", addr_space="Shared")
            h3fB = dram.tile([NC * HIB, 256], BF16, name="h3_fullB", addr_space="Shared")
            transform_bias(uT["v3"], w_sb["dW1"], ebfdW1, (h3sA[:], h3sB[:]), dinv2_sb)
            nc.gpsimd.collective_compute(
                "AllGather", mybir.AluOpType.bypass,
                replica_groups=[list(range(NC))],
                ins=[h3sA.opt()], outs=[h3fA.opt()])
            nc.gpsimd.collective_compute(
                "AllGather", mybir.AluOpType.bypass,
                replica_groups=[list(range(NC))],
                ins=[h3sB.opt()], outs=[h3fB.opt()])
            spmm(3, h3fA, h3fB, h3sA[:], h3sB[:], 256, w_sb["db1_row"], [uT["u4a"], uT["u4b"]], relu=True)

            # L4: h4' = u4 @ dW2, scale dinv2
            h4sA = dram.tile([LOA, 128], BF16, name="h4_shardA")
            h4sB = dram.tile([HIB, 128], BF16, name="h4_shardB")
            h4fA = dram.tile([NC * LOA, 128], BF16, name="h4_fullA", addr_space="Shared")
            h4fB = dram.tile([NC * HIB, 128], BF16, name="h4_fullB", addr_space="Shared")
            transform(4, [(uT["u4a"], 128), (uT["u4b"], 128)],
                      [dW2a, dW2b],
                      (h4sA[:], h4sB[:]), dinv2_sb, 2)
            nc.gpsimd.collective_compute(
                "AllGather", mybir.AluOpType.bypass,
                replica_groups=[list(range(NC))],
                ins=[h4sA.opt()], outs=[h4fA.opt()])
            nc.gpsimd.collective_compute(
                "AllGather", mybir.AluOpType.bypass,
                replica_groups=[list(range(NC))],
                ins=[h4sB.opt()], outs=[h4fB.opt()])
            spmm(4, h4fA, h4fB, h4sA[:], h4sB[:], 128, w_sb["db2_row"], [uT["f"]], relu=False)

            # final: xhat = (dinv * (f + sqrtdeg x db... )) @ dWf + dbf
            with tc.tile_pool(name="f_ps", bufs=2, space="PSUM") as fps, \
                 tc.tile_pool(name="f_sb", bufs=3) as fsb:
                for nt in range(NT):
                    for cb in range(2):
                        pf = fps.tile([128, 512], F32, tag="pf")
                        nc.tensor.matmul(pf[:],
                                         uT["f"][:, nt * 128:(nt + 1) * 128],
                                         w_sb["dWf"][:, cb * 512:(cb + 1) * 512],
                                         start=True, stop=False)
                        nc.tensor.matmul(pf[:],
                                         sqd_sb[0:1, nt * 128:(nt + 1) * 128],
                                         w_sb["dbf_row"][0:1, cb * 512:(cb + 1) * 512],
                                         start=False, stop=True)
                        ob = fsb.tile([128, 512], F32, tag="ob")
                        nc.scalar.activation(ob[:], pf[:],
                                             mybir.ActivationFunctionType.Copy,
                                             scale=dinv_sb[:, nt:nt + 1])
                        nc.sync.dma_start(out_d[cb, nt * 128:(nt + 1) * 128, :], ob[:])

    nc.finalize()
    return nc


# ---------------------------------------------------------------- entry point

def kernel(x, edge_index, eW1, eb1, eW2, eb2, eWf, ebf,
           dW1, db1, dW2, db2, dWf, dbf):
    x = np.asarray(x, dtype=np.float32)
    edge_index = np.asarray(edge_index)

    pre = preprocess(edge_index)
    w = pad_weights(eW1, eb1, eW2, eb2, eWf, ebf, dW1, db1, dW2, db2, dWf, dbf)
    in_maps = make_inmaps(pre, x, w)
    nc = build_program(pre["T"], pre["offs"], pre["TT"])

    trace = os.environ.get("GCAE_TRACE", "0") == "1"
    if trace:
        trace = _install_profile_hook()
    res = None
    last_err = None
    for attempt in range(3):
        try:
            res = run_bass_kernel_spmd(nc, in_maps, core_ids=list(range(NC)),
                                       trace=trace and attempt == 0)
            break
        except Exception as e:  # transient device wedge: retry, drop tracing
            last_err = e
    if res is None:
        raise last_err
    if trace and res.exec_time_ns:
        print(f"HW exec time: {res.exec_time_ns} ns")

    xhat_pad = np.empty((NPAD, 1024), dtype=np.float32)
    for c in range(NC):
        o = np.asarray(res.results[c]["xhat"])
        xhat_pad[c * NPC:(c + 1) * NPC, 0:512] = o[0]
        xhat_pad[c * NPC:(c + 1) * NPC, 512:1024] = o[1]
    return xhat_pad[pre["node_new"]]



# revision 11
# speedup vs baseline: 4.0764x; 1.2952x over previous
"""Trainium2 Bass kernel for a 6-layer GCN autoencoder (50k nodes, 800k edges).

Self-contained: host-side graph preprocessing (node permutation/packing,
edge binning, degree norms), an 8-core SPMD Bass/Tile program (gather-first
dataflow, one-hot scatter matmuls, dma_gather striped over all 4 SWDGE
queues, AllGather collectives), and output assembly.

Layer plumbing (per-node norm d = deg^-1/2, s = deg^1/2, so d*s = 1):
  L1 gathers host-staged xs = d*x directly (no transform, no AllGather):
      a1 = d * (sum_in xs[src] + xs[self])            == Ahat x
      u2 = relu(eW1^T a1 + eb1)                       == h1 (exact)
  L2: h2' = d*(u2 eW2p) node-major -> AllGather -> spmm with dst-side
      sqd x eb2p bias -> w2 (raw);   z-lite: zrow = d^2*(w2 eWfp + s x ebf)
      == d*z node-major -> AllGather
  L3: a3 = d * (sum zrow[src] + zrow[self]); u4 = relu(dW1p^T a3 + db1) == h3
  L4: h4' = d*(u4 dW2) -> AllGather -> spmm w/ sqd x db2 -> f (raw)
  out: xhat = d*(f dWf + s x dbf), written bf16, host-cast to f32.
"""
import sys
sys.path.insert(0, '/opt/trn_rl_repo')

import contextlib
import ctypes
import os
import types

import numpy as np
import ml_dtypes

import concourse.bacc as bacc
import concourse.bass as bass
import concourse.mybir as mybir
import concourse.tile as tile
from concourse.library_config import mlp
from concourse.vector_clock import ScopedClock
from concourse.bass_utils import run_bass_kernel_spmd


# ---- workaround: this walrus build rejects >2 sync waits on one instruction;
# spread Tile's tail-drain waits across single-wait SP NOPs.
def _patched_drain_and_barrier(self, tick_clock, wait_clock):
    nc = self.nc
    probe = nc.sync.nop()
    wait_clock.add_sem_waits(probe.ins, ScopedClock({None: tick_clock.global_clock}))
    si = probe.ins.sync_info
    waits = list(si.on_wait) if si is not None else []
    if si is not None:
        while si.on_wait:
            si.on_wait.pop()
    for w in waits:
        n = nc.sync.nop()
        n.ins.sync_info = mybir.SyncInfo(on_wait=[w], on_update=[])
    nc.sync.drain()
    nc.all_engine_barrier()
    assert self.sems is not None
    popped = nc._tile_sem_poison_stack.pop()
    assert popped is self._sem_poison
    nc.clear_and_free_semaphores(list(self.sems.allocated().values()))
    nc.all_engine_barrier()


tile.TileContext._drain_and_barrier = _patched_drain_and_barrier


# ---- optional NTFF profiling hook (GCAE_TRACE=1)
def _install_profile_hook():
    try:
        import antenv
    except ImportError:
        return False
    if getattr(antenv, "axon_hooks", None) is not None:
        return True
    so_path = "/opt/axon/libaxon_pjrt.so"
    if not os.path.exists(so_path):
        return False
    lib = ctypes.CDLL(so_path)
    if not hasattr(lib, "axon_start_nrt_profile"):
        return False
    lib.axon_start_nrt_profile.argtypes = [ctypes.POINTER(ctypes.c_int64), ctypes.c_size_t]
    lib.axon_start_nrt_profile.restype = ctypes.c_int64
    lib.axon_stop_nrt_profile.argtypes = [ctypes.c_char_p]
    lib.axon_stop_nrt_profile.restype = ctypes.c_int64

    @contextlib.contextmanager
    def _hook(output_dir, device_ids):
        import jax
        jax.devices()
        if device_ids:
            ids = (ctypes.c_int64 * len(device_ids))(*device_ids)
            rc = lib.axon_start_nrt_profile(ids, len(device_ids))
        else:
            rc = lib.axon_start_nrt_profile(None, 0)
        if rc != 0:
            raise RuntimeError(f"axon_start_nrt_profile rc={rc}")
        try:
            yield
        finally:
            n = lib.axon_stop_nrt_profile(str(output_dir).encode())
            if n < 0:
                raise RuntimeError(f"axon_stop_nrt_profile rc={n}")

    hooks = types.ModuleType("antenv.axon_hooks")
    _h = [_hook]
    hooks.set_axon_ntff_profile_hook = lambda h: _h.__setitem__(0, h)
    hooks.get_axon_ntff_profile_hook = lambda: _h[0]
    sys.modules["antenv.axon_hooks"] = hooks
    antenv.axon_hooks = hooks
    return True

F32 = mybir.dt.float32
BF16 = mybir.dt.bfloat16
I16 = mybir.dt.int16

N = 50000
NC = 8
BLK = 128
BPC = 49                 # blocks per core
NPC = BPC * BLK          # 6272 nodes per core
NPAD = NC * NPC          # 50176
HALF = NPAD // 2         # 25088
NT = BPC                 # node tiles per core
LOB = 24                 # blocks per core in sub-shard A
LOA = LOB * BLK          # 3072 rows/core in sub-shard A
HIB = NPC - LOA          # 3200 rows/core in sub-shard B


# ---------------------------------------------------------------- host prep

def preprocess(edge_index):
    src = np.asarray(edge_index[0], dtype=np.int64)
    dst = np.asarray(edge_index[1], dtype=np.int64)

    # degree includes the self-loops even though they are not in the edge
    # stream (they are applied on-device via an identity matmul)
    deg = (np.bincount(dst, minlength=N) + 1).astype(np.float64)
    dinv = np.where(deg > 0, 1.0 / np.sqrt(deg), 0.0)
    sqrtdeg = np.where(deg > 0, np.sqrt(deg), 0.0)

    # snake-deal nodes (sorted by degree desc) into 392 blocks
    nblocks = NC * BPC
    order = np.argsort(-deg, kind="stable")
    node_new = np.empty(N, dtype=np.int64)
    counts = np.zeros(nblocks, dtype=np.int64)
    bi = 0
    direction = 1
    for i, nd in enumerate(order):
        b = bi if direction == 1 else nblocks - 1 - bi
        node_new[nd] = b * BLK + counts[b]
        counts[b] += 1
        bi += 1
        if bi == nblocks:
            bi = 0
            direction = -direction
    assert counts.max() <= BLK

    # Edge halves (A/B sub-shard of the SRC) are fixed by which sub-shard a
    # node sits in; repacking nodes WITHIN a (core, sub-shard) never flips
    # any edge's half. Two host-only refinements under that invariant:
    # 1) 2D repack: redistribute nodes among their sub-shard's blocks to
    #    balance per-block (lo, hi) in-edge counts.
    # 2) slot alignment: within each sub-shard, order each core's blocks by
    #    lo-count so slot b holds similarly-sized groups on every core (the
    #    shared tile schedule takes max-over-cores per slot).
    s_p0 = node_new[src]
    d_p0 = node_new[dst]
    hf0 = ((s_p0 % NPC) >= LOA).astype(np.int64)
    node_lo = np.bincount(d_p0[hf0 == 0], minlength=NPAD).astype(np.int64)
    node_hi = np.bincount(d_p0[hf0 == 1], minlength=NPAD).astype(np.int64)

    new_pos = np.empty(NPAD, dtype=np.int64)
    for c in range(NC):
        for sh, (b0, nb) in enumerate(((0, LOB), (LOB, BPC - LOB))):
            rows = np.arange(c * NPC + b0 * BLK, c * NPC + (b0 + nb) * BLK)
            lo, hi = node_lo[rows], node_hi[rows]
            order = np.argsort(-(lo + hi), kind="stable")
            losum = np.zeros(nb); hisum = np.zeros(nb)
            nfill = np.zeros(nb, dtype=np.int64)
            assign = np.empty(len(rows), dtype=np.int64)
            for i in order:
                cost = (losum + lo[i]) ** 2 + (hisum + hi[i]) ** 2
                cost[nfill >= BLK] = np.inf
                j = int(np.argmin(cost))
                assign[i] = j
                losum[j] += lo[i]; hisum[j] += hi[i]; nfill[j] += 1
            slot_of = np.empty(nb, dtype=np.int64)
            slot_of[np.argsort(-losum, kind="stable")] = np.arange(nb)
            fill2 = np.zeros(nb, dtype=np.int64)
            for i in range(len(rows)):
                j = slot_of[assign[i]]
                new_pos[rows[i]] = c * NPC + (b0 + j) * BLK + fill2[j]
                fill2[j] += 1
    node_new = new_pos[node_new]

    s_p = node_new[src]
    d_p = node_new[dst]

    # per (core, block, half) edge lists
    core = d_p // NPC
    blk = (d_p % NPC) // BLK
    dloc = d_p % BLK
    score = s_p // NPC
    w = s_p % NPC
    hf = (w >= LOA).astype(np.int64)
    idxh = np.where(hf == 0, score * LOA + w, score * HIB + (w - LOA))

    cnt = np.zeros((NC, BPC, 2), dtype=np.int64)
    np.add.at(cnt, (core, blk, hf), 1)
    T = np.maximum(1, np.ceil(cnt.max(axis=0) / BLK).astype(np.int64))  # [BPC, 2]

    offs = np.zeros((BPC, 2), dtype=np.int64)   # tile offset of each (b, hf) group
    t = 0
    for b in range(BPC):
        for h in range(2):
            offs[b, h] = t
            t += T[b, h]
    TT = t                                       # total tiles per core

    idx_all = np.full((NC, TT * BLK), -1, dtype=np.int16)
    dloc_all = np.full((NC, TT * BLK), -1.0, dtype=np.float32)
    key = (core * BPC + blk) * 2 + hf
    ordkey = np.lexsort((idxh, key))
    ks = key[ordkey]
    sc, sb, sh = core[ordkey], blk[ordkey], hf[ordkey]
    si, sd = idxh[ordkey], dloc[ordkey]
    ne = len(ks)
    starts = np.r_[0, np.flatnonzero(np.diff(ks)) + 1]
    glen = np.diff(np.r_[starts, ne])
    pos = np.arange(ne) - np.repeat(starts, glen)
    slot = offs[sb, sh] * BLK + pos
    idx_all[sc, slot] = si.astype(np.int16)
    dloc_all[sc, slot] = sd.astype(np.float32)

    sd_pad = np.zeros(NPAD, dtype=np.float32)
    di_pad = np.zeros(NPAD, dtype=np.float32)
    sd_pad[node_new] = sqrtdeg
    di_pad[node_new] = dinv

    return dict(node_new=node_new, T=T, offs=offs, TT=TT, cnt=cnt,
                idx_all=idx_all, dloc_all=dloc_all,
                sqrtdeg=sd_pad, dinv=di_pad)


def make_inmaps(pre, x, weights):
    """weights: dict of padded bf16 weight/bias arrays (shared across cores)."""
    node_new = pre["node_new"]
    TT = pre["TT"]
    Tmax = int(pre["T"].max())
    bf = ml_dtypes.bfloat16

    # host-staged, dinv-prescaled node-major x (the L1 gather source)
    xs = np.zeros((NPAD, 128), dtype=np.float32)
    xs[node_new] = np.asarray(x, dtype=np.float32)
    xs *= pre["dinv"][:, None]
    xs3 = xs.reshape(NC, NPC, 128)
    xsA = np.ascontiguousarray(xs3[:, :LOA, :].reshape(NC * LOA, 128)).astype(bf)
    xsB = np.ascontiguousarray(xs3[:, LOA:, :].reshape(NC * HIB, 128)).astype(bf)

    in_maps = []
    for c in range(NC):
        m = {}
        m["xsA"] = xsA
        m["xsB"] = xsB
        m["xloc"] = np.ascontiguousarray(xs3[c]).astype(bf)    # [NPC, 128]
        idx = pre["idx_all"][c]
        m["idxs"] = np.tile(idx.reshape(TT * 8, 16).T, (8, 1)).copy()
        dl = pre["dloc_all"][c].reshape(TT, BLK).T             # [128, TT]
        m["dstloc"] = np.ascontiguousarray(dl, dtype=bf)
        sl = slice(c * NPC, (c + 1) * NPC)
        gc = pre["cnt"][c].T.reshape(1, 2 * BPC)               # [1, 2*BPC], hf-major
        m["gcount"] = np.ascontiguousarray(gc, dtype=np.int32)
        m["sqrtdeg_row"] = pre["sqrtdeg"][sl][None, :].astype(bf)
        m["dinv_col"] = pre["dinv"][sl].reshape(BPC, BLK).T.astype(np.float32).copy()
        m["dinv2_col"] = (pre["dinv"][sl] ** 2).reshape(BPC, BLK).T.astype(np.float32).copy()
        m["dinvb"] = np.tile(pre["dinv"][sl][None, :], (128, 1)).astype(bf)
        m["one_row"] = np.ones((1, 128), dtype=np.float32).astype(bf)
        R = np.tile(np.arange(BLK, dtype=np.float32), (128, Tmax)).astype(bf)
        m["Rbig"] = R
        m["ident"] = np.eye(128, dtype=np.float32).astype(bf)
        m.update(weights)
        in_maps.append(m)
    return in_maps


def pad_weights(eW1, eb1, eW2, eb2, eWf, ebf, dW1, db1, dW2, db2, dWf, dbf):
    bf = ml_dtypes.bfloat16
    w = {}
    w["eW1"] = np.asarray(eW1, np.float32).astype(bf)                       # [128,128]
    eW2p = np.zeros((128, 128), np.float32); eW2p[:, :64] = eW2
    w["eW2p"] = eW2p.astype(bf)
    eWfp = np.zeros((128, 128), np.float32); eWfp[:64, :64] = eWf
    w["eWfp"] = eWfp.astype(bf)                                             # [128,128]
    dW1p = np.zeros((128, 256), np.float32); dW1p[:64] = dW1
    w["dW1p"] = dW1p.astype(bf)                                             # [128,256]
    w["dW2"] = np.asarray(dW2, np.float32).astype(bf)                       # [256,128]
    w["dWf"] = np.asarray(dWf, np.float32).astype(bf)                       # [128,1024]
    w["eb1_row"] = np.asarray(eb1, np.float32)[None, :].astype(bf)          # [1,128]
    eb2r = np.zeros((1, 128), np.float32); eb2r[0, :64] = eb2
    w["eb2p_row"] = eb2r.astype(bf)
    ebfr = np.zeros((1, 128), np.float32); ebfr[0, :64] = ebf
    w["ebf_row"] = ebfr.astype(bf)                                          # [1,128]
    w["db1_row"] = np.asarray(db1, np.float32)[None, :].astype(bf)          # [1,256]
    w["db2_row"] = np.asarray(db2, np.float32)[None, :].astype(bf)          # [1,128]
    w["dbf_row"] = np.asarray(dbf, np.float32)[None, :].astype(bf)          # [1,1024]
    return w


# ---------------------------------------------------------------- device program

def build_program(T, offs, TT):
    """T: [BPC,2] tiles per (block,half); offs: [BPC,2] tile offsets; TT total tiles."""
    Tmax = int(T.max())
    nc = bacc.Bacc(None, target_bir_lowering=False, num_swdge_queues=4)

    # ---- I/O
    xsA_d = nc.dram_tensor("xsA", [NC * LOA, 128], BF16, kind="ExternalInput")
    xsB_d = nc.dram_tensor("xsB", [NC * HIB, 128], BF16, kind="ExternalInput")
    xloc_d = nc.dram_tensor("xloc", [NPC, 128], BF16, kind="ExternalInput")
    idx_d = nc.dram_tensor("idxs", [128, TT * 8], I16, kind="ExternalInput")
    dloc_d = nc.dram_tensor("dstloc", [128, TT], BF16, kind="ExternalInput")
    gcount_d = nc.dram_tensor("gcount", [1, 2 * BPC], mybir.dt.int32, kind="ExternalInput")
    sqd_d = nc.dram_tensor("sqrtdeg_row", [1, NPC], BF16, kind="ExternalInput")
    dinv_d = nc.dram_tensor("dinv_col", [128, BPC], F32, kind="ExternalInput")
    dinv2_d = nc.dram_tensor("dinv2_col", [128, BPC], F32, kind="ExternalInput")
    dinvb_d = nc.dram_tensor("dinvb", [128, NPC], BF16, kind="ExternalInput")
    one_d = nc.dram_tensor("one_row", [1, 128], BF16, kind="ExternalInput")
    R_d = nc.dram_tensor("Rbig", [128, Tmax * 128], BF16, kind="ExternalInput")
    id_d = nc.dram_tensor("ident", [128, 128], BF16, kind="ExternalInput")
    wnames = {"eW1": [128, 128], "eW2p": [128, 128], "eWfp": [128, 128],
              "dW1p": [128, 256], "dW2": [256, 128], "dWf": [128, 1024],
              "eb1_row": [1, 128], "eb2p_row": [1, 128], "ebf_row": [1, 128],
              "db1_row": [1, 256], "db2_row": [1, 128], "dbf_row": [1, 1024]}
    w_d = {k: nc.dram_tensor(k, shp, BF16, kind="ExternalInput")
           for k, shp in wnames.items()}
    out_d = nc.dram_tensor("xhat", [2, NPC, 512], BF16, kind="ExternalOutput")

    with tile.TileContext(nc) as tc:
        with tc.tile_pool(name="const", bufs=1) as cpool, \
             tc.tile_pool(name="acts", bufs=1) as apool, \
             tc.tile_pool(name="dram", bufs=1, space="DRAM") as dram, \
             tc.tile_pool(name="wps", bufs=4, space="PSUM") as pps, \
             tc.tile_pool(name="wtr", bufs=4, space="PSUM") as ptr, \
             tc.tile_pool(name="wm", bufs=8) as pm, \
             tc.tile_pool(name="ws", bufs=4) as psl, \
             tc.tile_pool(name="wh", bufs=3) as ph, \
             tc.tile_pool(name="wn", bufs=4) as phn:
            nc.gpsimd.load_library(mlp)

            # ---- persistent SBUF state
            w_sb = {}
            for k, shp in wnames.items():
                if shp[0] > 128:
                    continue
                t = cpool.tile(shp, BF16, name=f"w_{k}")
                nc.sync.dma_start(t[:], w_d[k][:])
                w_sb[k] = t
            dW2a = cpool.tile([128, 128], BF16, name="w_dW2a")
            nc.sync.dma_start(dW2a[:], w_d["dW2"][0:128, :])
            dW2b = cpool.tile([128, 128], BF16, name="w_dW2b")
            nc.sync.dma_start(dW2b[:], w_d["dW2"][128:256, :])
            idx_sb = cpool.tile([128, TT * 8], I16, name="idx_sb")
            nc.sync.dma_start(idx_sb[:], idx_d[:])
            dloc_sb = cpool.tile([128, TT], BF16, name="dloc_sb")
            nc.sync.dma_start(dloc_sb[:], dloc_d[:])
            gcount_sb = cpool.tile([1, 2 * BPC], mybir.dt.int32, name="gcount_sb")
            nc.sync.dma_start(gcount_sb[:], gcount_d[:])
            sqd_sb = cpool.tile([1, NPC], BF16, name="sqd_sb")
            nc.sync.dma_start(sqd_sb[:], sqd_d[:])
            dinv_sb = cpool.tile([128, BPC], F32, name="dinv_sb")
            nc.sync.dma_start(dinv_sb[:], dinv_d[:])
            dinv2_sb = cpool.tile([128, BPC], F32, name="dinv2_sb")
            nc.sync.dma_start(dinv2_sb[:], dinv2_d[:])
            dinvb_sb = cpool.tile([128, NPC], BF16, name="dinvb_sb")
            nc.sync.dma_start(dinvb_sb[:], dinvb_d[:])
            one_sb = cpool.tile([1, 128], BF16, name="one_sb")
            nc.sync.dma_start(one_sb[:], one_d[:])
            R_sb = cpool.tile([128, Tmax * 128], BF16, name="R_sb")
            nc.sync.dma_start(R_sb[:], R_d[:])
            id_sb = cpool.tile([128, 128], BF16, name="id_sb")
            nc.sync.dma_start(id_sb[:], id_d[:])

            R3 = R_sb[:].rearrange("p (t d) -> p t d", d=128)

            uT = {}  # feature-major activation arrays per stage
            for nm in ("a1", "u2", "w2", "a3", "u4a", "u4b", "f"):
                uT[nm] = apool.tile([128, NPC], BF16, name=f"{nm}T")

            qstate = [0]

            # ---------------- SpMM: gather-aggregate into per-block PSUM
            # chains; pass A includes the self-loop identity mm (+ optional
            # dst-side sqd x bias mm); pass B re-injects the bf16 pass-A
            # partial via an identity matmul; per-block copy-out via copy_cb.
            def spmm(layer, bufA, bufB, locA, locB, bias_row, out_t, copy_cb):
                for hf in range(2):
                    buf = bufA if hf == 0 else bufB
                    for b in range(BPC):
                        Tb = int(T[b, hf]); off = int(offs[b, hf])
                        pb = pps.tile([128, 128], F32, tag="pb")
                        if hf == 0:
                            hblk = ph.tile([128, 128], BF16, tag="hblk")
                            if b < LOB:
                                nc.sync.dma_start(hblk[:], locA[b * 128:(b + 1) * 128, :])
                            else:
                                nc.sync.dma_start(hblk[:], locB[(b - LOB) * 128:(b - LOB + 1) * 128, :])
                            if bias_row is not None:
                                nc.tensor.matmul(
                                    pb[:], bias_row[0:1, :],
                                    sqd_sb[0:1, b * 128:(b + 1) * 128],
                                    start=True, stop=False)
                            nc.tensor.matmul(
                                pb[:], hblk[:], id_sb[:],
                                start=(bias_row is None), stop=False)
                        else:
                            nc.tensor.matmul(
                                pb[:], id_sb[:], out_t[:, b * 128:(b + 1) * 128],
                                start=True, stop=False)
                        msg = pm.tile([128, Tmax, 128], BF16, tag="msg")
                        if layer == 1 and hf == 0 and b < 8:
                            nc.vector.memset(msg[:], 0.0)
                        creg = nc.gpsimd.alloc_register()
                        nc.gpsimd.load(creg, gcount_sb[0:1, hf * BPC + b:hf * BPC + b + 1])
                        nc.gpsimd.dma_gather(
                            msg[:, :Tb, :], buf[:],
                            idx_sb[:, off * 8:(off + Tb) * 8],
                            Tb * 128, creg, 128, single_packet=False,
                            queue_num=qstate[0])
                        qstate[0] = (qstate[0] + 1) % 4
                        S = psl.tile([128, Tmax, 128], BF16, tag="S")
                        d3 = dloc_sb[:, off:off + Tb].broadcast_to([128, Tb, 128])
                        nc.vector.tensor_tensor(S[:, :Tb, :], R3[:, :Tb, :], d3,
                                                mybir.AluOpType.is_equal)
                        for t in range(Tb):
                            nc.tensor.matmul(
                                pb[:], msg[:, t, :], S[:, t, :],
                                start=False, stop=(t == Tb - 1))
                        osl = out_t[:, b * 128:(b + 1) * 128]
                        copy_cb(b, hf, pb, osl)

            def cb_plain(b, hf, pb, osl):
                nc.scalar.activation(osl, pb[:], mybir.ActivationFunctionType.Copy)

            def cb_dinvb(b, hf, pb, osl):
                if hf == 0:
                    nc.scalar.activation(osl, pb[:],
                                         mybir.ActivationFunctionType.Copy)
                else:
                    nc.vector.tensor_tensor(
                        osl, pb[:], dinvb_sb[:, b * 128:(b + 1) * 128],
                        mybir.AluOpType.mult)

            def cb_final(b, hf, pb, osl):
                nc.scalar.activation(osl, pb[:], mybir.ActivationFunctionType.Copy)
                if hf == 0:
                    return
                # final stage for block b: xhat = d*(f dWf + s x dbf), bf16 out
                for cb in range(2):
                    pf = ptr.tile([128, 512], F32, tag="tr")
                    nc.tensor.matmul(pf[:], osl,
                                     w_sb["dWf"][:, cb * 512:(cb + 1) * 512],
                                     start=True, stop=False)
                    nc.tensor.matmul(pf[:],
                                     sqd_sb[0:1, b * 128:(b + 1) * 128],
                                     w_sb["dbf_row"][0:1, cb * 512:(cb + 1) * 512],
                                     start=False, stop=True)
                    ob = phn.tile([128, 512], BF16, tag="ob")
                    nc.scalar.activation(ob[:], pf[:],
                                         mybir.ActivationFunctionType.Copy,
                                         scale=dinv_sb[:, b:b + 1])
                    nc.sync.dma_start(out_d[cb, b * 128:(b + 1) * 128, :], ob[:])

            # ---------------- feature-major "lite" transform:
            # out_fm = act(W^T @ in_fm + bias x ones)  per 128-node tile
            def tlite(in_t, Ws, bias_row, out_ts, act):
                nchb = len(out_ts)
                for nt in range(NT):
                    for chb in range(nchb):
                        pt = ptr.tile([128, 128], F32, tag="tr")
                        nc.tensor.matmul(pt[:], Ws[:, chb * 128:(chb + 1) * 128],
                                         in_t[:, nt * 128:(nt + 1) * 128],
                                         start=True, stop=False)
                        nc.tensor.matmul(pt[:], bias_row[0:1, chb * 128:(chb + 1) * 128],
                                         one_sb[0:1, :], start=False, stop=True)
                        nc.scalar.activation(
                            out_ts[chb][:, nt * 128:(nt + 1) * 128], pt[:], act)

            # ---------------- node-major transform + shard write:
            # shard rows = scale_col * (sum_k u_k^T @ W_k [+ s x bias])
            def transform_nm(parts, bias_row, shards, scale_col):
                shA, shB = shards
                for nt in range(NT):
                    hb = ptr.tile([128, 128], F32, tag="tr")
                    for ki, (ut, Wk) in enumerate(parts):
                        nc.tensor.matmul(hb[:], ut[:, nt * 128:(nt + 1) * 128],
                                         Wk[:], start=(ki == 0),
                                         stop=(bias_row is None and
                                               ki == len(parts) - 1))
                    if bias_row is not None:
                        nc.tensor.matmul(hb[:], sqd_sb[0:1, nt * 128:(nt + 1) * 128],
                                         bias_row[0:1, :], start=False, stop=True)
                    hn = phn.tile([128, 128], BF16, tag="hn")
                    nc.scalar.activation(hn[:], hb[:],
                                         mybir.ActivationFunctionType.Copy,
                                         scale=scale_col[:, nt:nt + 1])
                    if nt < LOB:
                        nc.sync.dma_start(shA[nt * 128:(nt + 1) * 128, :], hn[:])
                    else:
                        nc.sync.dma_start(shB[(nt - LOB) * 128:(nt - LOB + 1) * 128, :], hn[:])

            def mkshard(name, ch):
                sA = dram.tile([LOA, ch], BF16, name=f"{name}_shardA")
                sB = dram.tile([HIB, ch], BF16, name=f"{name}_shardB")
                fA = dram.tile([NC * LOA, ch], BF16, name=f"{name}_fullA", addr_space="Shared")
                fB = dram.tile([NC * HIB, ch], BF16, name=f"{name}_fullB", addr_space="Shared")
                return sA, sB, fA, fB

            def allgather(sX, fX):
                nc.gpsimd.collective_compute(
                    "AllGather", mybir.AluOpType.bypass,
                    replica_groups=[list(range(NC))],
                    ins=[sX.opt()], outs=[fX.opt()])

            # ================= network =================
            # L1: gather host-staged xs directly; a1 = d * (sum + self)
            spmm(1, xsA_d, xsB_d, xloc_d[0:LOA, :], xloc_d[LOA:NPC, :],
                 None, uT["a1"], cb_dinvb)
            # u2 = relu(eW1^T a1 + eb1) == h1
            tlite(uT["a1"], w_sb["eW1"], w_sb["eb1_row"], [uT["u2"]],
                  mybir.ActivationFunctionType.Relu)

            # L2: h2' = d*(u2 eW2p) node-major -> AllGather -> spmm (+eb2p)
            h2sA, h2sB, h2fA, h2fB = mkshard("h2", 128)
            transform_nm([(uT["u2"], w_sb["eW2p"])], None, (h2sA[:], h2sB[:]), dinv_sb)
            allgather(h2sA, h2fA)
            allgather(h2sB, h2fB)
            spmm(2, h2fA, h2fB, h2sA[:], h2sB[:], w_sb["eb2p_row"], uT["w2"], cb_plain)

            # z-lite: zrow = d^2*(w2 eWfp + s x ebf) == d*z (cols 64+ zero)
            h3sA, h3sB, h3fA, h3fB = mkshard("h3", 128)
            transform_nm([(uT["w2"], w_sb["eWfp"])], w_sb["ebf_row"],
                         (h3sA[:], h3sB[:]), dinv2_sb)
            allgather(h3sA, h3fA)
            allgather(h3sB, h3fB)
            # L3: a3 = d * (sum zrow + self)
            spmm(3, h3fA, h3fB, h3sA[:], h3sB[:], None, uT["a3"], cb_dinvb)
            # u4 = relu(dW1p^T a3 + db1) == h3
            tlite(uT["a3"], w_sb["dW1p"], w_sb["db1_row"], [uT["u4a"], uT["u4b"]],
                  mybir.ActivationFunctionType.Relu)

            # L4: h4' = d*(u4 dW2) -> AllGather -> spmm (+db2) -> final
            h4sA, h4sB, h4fA, h4fB = mkshard("h4", 128)
            transform_nm([(uT["u4a"], dW2a), (uT["u4b"], dW2b)], None,
                         (h4sA[:], h4sB[:]), dinv_sb)
            allgather(h4sA, h4fA)
            allgather(h4sB, h4fB)
            spmm(4, h4fA, h4fB, h4sA[:], h4sB[:], w_sb["db2_row"], uT["f"], cb_final)

    nc.finalize()
    return nc


# ---------------------------------------------------------------- entry point

def kernel(x, edge_index, eW1, eb1, eW2, eb2, eWf, ebf,
           dW1, db1, dW2, db2, dWf, dbf):
    x = np.asarray(x, dtype=np.float32)
    edge_index = np.asarray(edge_index)

    pre = preprocess(edge_index)
    w = pad_weights(eW1, eb1, eW2, eb2, eWf, ebf, dW1, db1, dW2, db2, dWf, dbf)
    in_maps = make_inmaps(pre, x, w)
    nc = build_program(pre["T"], pre["offs"], pre["TT"])

    trace = os.environ.get("GCAE_TRACE", "0") == "1"
    if trace:
        trace = _install_profile_hook()
    res = None
    last_err = None
    for attempt in range(3):
        try:
            res = run_bass_kernel_spmd(nc, in_maps, core_ids=list(range(NC)),
                                       trace=trace and attempt == 0)
            break
        except Exception as e:  # transient device wedge: retry, drop tracing
            last_err = e
    if res is None:
        raise last_err
    if trace and res.exec_time_ns:
        print(f"HW exec time: {res.exec_time_ns} ns")

    xhat_pad = np.empty((NPAD, 1024), dtype=np.float32)
    for c in range(NC):
        o = np.asarray(res.results[c]["xhat"]).astype(np.float32)
        xhat_pad[c * NPC:(c + 1) * NPC, 0:512] = o[0]
        xhat_pad[c * NPC:(c + 1) * NPC, 512:1024] = o[1]
    return xhat_pad[pre["node_new"]]


# revision 15
# speedup vs baseline: 4.7101x; 1.1555x over previous
"""Trainium2 Bass kernel for a 6-layer GCN autoencoder (50k nodes, 800k edges).

Self-contained: host-side graph preprocessing (node permutation/packing,
edge binning, degree norms), an 8-core SPMD Bass/Tile program (gather-first
dataflow, one-hot scatter matmuls, dma_gather striped over all 4 SWDGE
queues, AllGather collectives), and output assembly.

Layer plumbing (per-node norm d = deg^-1/2, s = deg^1/2, so d*s = 1):
  L1 gathers host-staged xs = d*x directly (no transform, no AllGather):
      a1 = d * (sum_in xs[src] + xs[self])            == Ahat x
      u2 = relu(eW1^T a1 + eb1)                       == h1 (exact)
  L2: h2' = d*(u2 eW2p) node-major -> AllGather -> spmm with dst-side
      sqd x eb2p bias -> w2 (raw);   z-lite: zrow = d^2*(w2 eWfp + s x ebf)
      == d*z node-major -> AllGather
  L3: a3 = d * (sum zrow[src] + zrow[self]); u4 = relu(dW1p^T a3 + db1) == h3
  L4: h4' = d*(u4 dW2) -> AllGather -> spmm w/ sqd x db2 -> f (raw)
  out: xhat = d*(f dWf + s x dbf), written bf16, host-cast to f32.
"""
import sys
sys.path.insert(0, '/opt/trn_rl_repo')

import contextlib
import ctypes
import os
import types

import numpy as np
import ml_dtypes

import concourse.bacc as bacc
import concourse.bass as bass
import concourse.mybir as mybir
import concourse.tile as tile
from concourse.library_config import mlp
from concourse.vector_clock import ScopedClock
from concourse.bass_utils import run_bass_kernel_spmd


# ---- workaround: this walrus build rejects >2 sync waits on one instruction;
# spread Tile's tail-drain waits across single-wait SP NOPs.
def _patched_drain_and_barrier(self, tick_clock, wait_clock):
    nc = self.nc
    probe = nc.sync.nop()
    wait_clock.add_sem_waits(probe.ins, ScopedClock({None: tick_clock.global_clock}))
    si = probe.ins.sync_info
    waits = list(si.on_wait) if si is not None else []
    if si is not None:
        while si.on_wait:
            si.on_wait.pop()
    for w in waits:
        n = nc.sync.nop()
        n.ins.sync_info = mybir.SyncInfo(on_wait=[w], on_update=[])
    nc.sync.drain()
    nc.all_engine_barrier()
    assert self.sems is not None
    popped = nc._tile_sem_poison_stack.pop()
    assert popped is self._sem_poison
    nc.clear_and_free_semaphores(list(self.sems.allocated().values()))
    nc.all_engine_barrier()


tile.TileContext._drain_and_barrier = _patched_drain_and_barrier


# ---- optional NTFF profiling hook (GCAE_TRACE=1)
def _install_profile_hook():
    try:
        import antenv
    except ImportError:
        return False
    if getattr(antenv, "axon_hooks", None) is not None:
        return True
    so_path = "/opt/axon/libaxon_pjrt.so"
    if not os.path.exists(so_path):
        return False
    lib = ctypes.CDLL(so_path)
    if not hasattr(lib, "axon_start_nrt_profile"):
        return False
    lib.axon_start_nrt_profile.argtypes = [ctypes.POINTER(ctypes.c_int64), ctypes.c_size_t]
    lib.axon_start_nrt_profile.restype = ctypes.c_int64
    lib.axon_stop_nrt_profile.argtypes = [ctypes.c_char_p]
    lib.axon_stop_nrt_profile.restype = ctypes.c_int64

    @contextlib.contextmanager
    def _hook(output_dir, device_ids):
        import jax
        jax.devices()
        if device_ids:
            ids = (ctypes.c_int64 * len(device_ids))(*device_ids)
            rc = lib.axon_start_nrt_profile(ids, len(device_ids))
        else:
            rc = lib.axon_start_nrt_profile(None, 0)
        if rc != 0:
            raise RuntimeError(f"axon_start_nrt_profile rc={rc}")
        try:
            yield
        finally:
            n = lib.axon_stop_nrt_profile(str(output_dir).encode())
            if n < 0:
                raise RuntimeError(f"axon_stop_nrt_profile rc={n}")

    hooks = types.ModuleType("antenv.axon_hooks")
    _h = [_hook]
    hooks.set_axon_ntff_profile_hook = lambda h: _h.__setitem__(0, h)
    hooks.get_axon_ntff_profile_hook = lambda: _h[0]
    sys.modules["antenv.axon_hooks"] = hooks
    antenv.axon_hooks = hooks
    return True

F32 = mybir.dt.float32
BF16 = mybir.dt.bfloat16
I16 = mybir.dt.int16

N = 50000
NC = 8
BLK = 128
BPC = 49                 # blocks per core
NPC = BPC * BLK          # 6272 nodes per core
NPAD = NC * NPC          # 50176
HALF = NPAD // 2         # 25088
NT = BPC                 # node tiles per core
LOB = 24                 # blocks per core in sub-shard A
LOA = LOB * BLK          # 3072 rows/core in sub-shard A
HIB = NPC - LOA          # 3200 rows/core in sub-shard B


# ---------------------------------------------------------------- host prep

def preprocess(edge_index):
    src = np.asarray(edge_index[0], dtype=np.int64)
    dst = np.asarray(edge_index[1], dtype=np.int64)

    # degree includes the self-loops even though they are not in the edge
    # stream (they are applied on-device via an identity matmul)
    deg = (np.bincount(dst, minlength=N) + 1).astype(np.float64)
    dinv = np.where(deg > 0, 1.0 / np.sqrt(deg), 0.0)
    sqrtdeg = np.where(deg > 0, np.sqrt(deg), 0.0)

    # snake-deal nodes (sorted by degree desc) into 392 blocks
    nblocks = NC * BPC
    order = np.argsort(-deg, kind="stable")
    node_new = np.empty(N, dtype=np.int64)
    counts = np.zeros(nblocks, dtype=np.int64)
    bi = 0
    direction = 1
    for i, nd in enumerate(order):
        b = bi if direction == 1 else nblocks - 1 - bi
        node_new[nd] = b * BLK + counts[b]
        counts[b] += 1
        bi += 1
        if bi == nblocks:
            bi = 0
            direction = -direction
    assert counts.max() <= BLK

    # Edge halves (A/B sub-shard of the SRC) are fixed by which sub-shard a
    # node sits in; repacking nodes WITHIN a (core, sub-shard) never flips
    # any edge's half. Two host-only refinements under that invariant:
    # 1) 2D repack: redistribute nodes among their sub-shard's blocks to
    #    balance per-block (lo, hi) in-edge counts.
    # 2) slot alignment: within each sub-shard, order each core's blocks by
    #    lo-count so slot b holds similarly-sized groups on every core (the
    #    shared tile schedule takes max-over-cores per slot).
    s_p0 = node_new[src]
    d_p0 = node_new[dst]
    hf0 = ((s_p0 % NPC) >= LOA).astype(np.int64)
    node_lo = np.bincount(d_p0[hf0 == 0], minlength=NPAD).astype(np.int64)
    node_hi = np.bincount(d_p0[hf0 == 1], minlength=NPAD).astype(np.int64)

    new_pos = np.empty(NPAD, dtype=np.int64)
    for c in range(NC):
        for sh, (b0, nb) in enumerate(((0, LOB), (LOB, BPC - LOB))):
            rows = np.arange(c * NPC + b0 * BLK, c * NPC + (b0 + nb) * BLK)
            lo, hi = node_lo[rows], node_hi[rows]
            order = np.argsort(-(lo + hi), kind="stable")
            losum = np.zeros(nb); hisum = np.zeros(nb)
            nfill = np.zeros(nb, dtype=np.int64)
            assign = np.empty(len(rows), dtype=np.int64)
            for i in order:
                cost = (losum + lo[i]) ** 2 + (hisum + hi[i]) ** 2
                cost[nfill >= BLK] = np.inf
                j = int(np.argmin(cost))
                assign[i] = j
                losum[j] += lo[i]; hisum[j] += hi[i]; nfill[j] += 1
            slot_of = np.empty(nb, dtype=np.int64)
            slot_of[np.argsort(-losum, kind="stable")] = np.arange(nb)
            fill2 = np.zeros(nb, dtype=np.int64)
            for i in range(len(rows)):
                j = slot_of[assign[i]]
                new_pos[rows[i]] = c * NPC + (b0 + j) * BLK + fill2[j]
                fill2[j] += 1
    node_new = new_pos[node_new]

    s_p = node_new[src]
    d_p = node_new[dst]

    # per (core, block, half) edge lists
    core = d_p // NPC
    blk = (d_p % NPC) // BLK
    dloc = d_p % BLK
    score = s_p // NPC
    w = s_p % NPC
    hf = (w >= LOA).astype(np.int64)
    idxh = np.where(hf == 0, score * LOA + w, score * HIB + (w - LOA))

    cnt = np.zeros((NC, BPC, 2), dtype=np.int64)
    np.add.at(cnt, (core, blk, hf), 1)
    T = np.maximum(1, np.ceil(cnt.max(axis=0) / BLK).astype(np.int64))  # [BPC, 2]

    offs = np.zeros((BPC, 2), dtype=np.int64)   # tile offset of each (b, hf) group
    t = 0
    for b in range(BPC):
        for h in range(2):
            offs[b, h] = t
            t += T[b, h]
    TT = t                                       # total tiles per core

    idx_all = np.full((NC, TT * BLK), -1, dtype=np.int16)
    dloc_all = np.full((NC, TT * BLK), -1.0, dtype=np.float32)
    key = (core * BPC + blk) * 2 + hf
    ordkey = np.lexsort((idxh, key))
    ks = key[ordkey]
    sc, sb, sh = core[ordkey], blk[ordkey], hf[ordkey]
    si, sd = idxh[ordkey], dloc[ordkey]
    ne = len(ks)
    starts = np.r_[0, np.flatnonzero(np.diff(ks)) + 1]
    glen = np.diff(np.r_[starts, ne])
    pos = np.arange(ne) - np.repeat(starts, glen)
    slot = offs[sb, sh] * BLK + pos
    idx_all[sc, slot] = si.astype(np.int16)
    dloc_all[sc, slot] = sd.astype(np.float32)

    sd_pad = np.zeros(NPAD, dtype=np.float32)
    di_pad = np.zeros(NPAD, dtype=np.float32)
    sd_pad[node_new] = sqrtdeg
    di_pad[node_new] = dinv

    return dict(node_new=node_new, T=T, offs=offs, TT=TT, cnt=cnt,
                idx_all=idx_all, dloc_all=dloc_all,
                sqrtdeg=sd_pad, dinv=di_pad)


def make_inmaps(pre, x, weights):
    """weights: dict of padded bf16 weight/bias arrays (shared across cores)."""
    node_new = pre["node_new"]
    TT = pre["TT"]
    Tmax = int(pre["T"].max())
    bf = ml_dtypes.bfloat16

    # host-staged, dinv-prescaled node-major x (the L1 gather source)
    xs = np.zeros((NPAD, 128), dtype=np.float32)
    xs[node_new] = np.asarray(x, dtype=np.float32)
    xs *= pre["dinv"][:, None]
    xs3 = xs.reshape(NC, NPC, 128)
    xsA = np.ascontiguousarray(xs3[:, :LOA, :].reshape(NC * LOA, 128)).astype(bf)
    xsB = np.ascontiguousarray(xs3[:, LOA:, :].reshape(NC * HIB, 128)).astype(bf)

    in_maps = []
    for c in range(NC):
        m = {}
        m["xsA"] = xsA
        m["xsB"] = xsB
        m["xloc"] = np.ascontiguousarray(xs3[c]).astype(bf)    # [NPC, 128]
        idx = pre["idx_all"][c]
        m["idxs"] = np.tile(idx.reshape(TT * 8, 16).T, (8, 1)).copy()
        dl = pre["dloc_all"][c].reshape(TT, BLK).T             # [128, TT]
        m["dstloc"] = np.ascontiguousarray(dl, dtype=bf)
        sl = slice(c * NPC, (c + 1) * NPC)
        gc = pre["cnt"][c].T.reshape(1, 2 * BPC)               # [1, 2*BPC], hf-major
        m["gcount"] = np.ascontiguousarray(gc, dtype=np.int32)
        m["sqrtdeg_row"] = pre["sqrtdeg"][sl][None, :].astype(bf)
        m["dinv_col"] = pre["dinv"][sl].reshape(BPC, BLK).T.astype(np.float32).copy()
        m["dinv2_col"] = (pre["dinv"][sl] ** 2).reshape(BPC, BLK).T.astype(np.float32).copy()
        m["dinvb"] = np.tile(pre["dinv"][sl][None, :], (128, 1)).astype(bf)
        m["one_row"] = np.ones((1, 128), dtype=np.float32).astype(bf)
        R = np.tile(np.arange(BLK, dtype=np.float32), (128, Tmax)).astype(bf)
        m["Rbig"] = R
        m["ident"] = np.eye(128, dtype=np.float32).astype(bf)
        m.update(weights)
        in_maps.append(m)
    return in_maps


def pad_weights(eW1, eb1, eW2, eb2, eWf, ebf, dW1, db1, dW2, db2, dWf, dbf):
    bf = ml_dtypes.bfloat16
    w = {}
    w["eW1"] = np.asarray(eW1, np.float32).astype(bf)                       # [128,128]
    eW2p = np.zeros((128, 128), np.float32); eW2p[:, :64] = eW2
    w["eW2p"] = eW2p.astype(bf)
    eWfp = np.zeros((128, 128), np.float32); eWfp[:64, :64] = eWf
    w["eWfp"] = eWfp.astype(bf)                                             # [128,128]
    dW1p = np.zeros((128, 256), np.float32); dW1p[:64] = dW1
    w["dW1p"] = dW1p.astype(bf)                                             # [128,256]
    w["dW2"] = np.asarray(dW2, np.float32).astype(bf)                       # [256,128]
    w["dWf"] = np.asarray(dWf, np.float32).astype(bf)                       # [128,1024]
    w["eb1_row"] = np.asarray(eb1, np.float32)[None, :].astype(bf)          # [1,128]
    eb2r = np.zeros((1, 128), np.float32); eb2r[0, :64] = eb2
    w["eb2p_row"] = eb2r.astype(bf)
    ebfr = np.zeros((1, 128), np.float32); ebfr[0, :64] = ebf
    w["ebf_row"] = ebfr.astype(bf)                                          # [1,128]
    w["db1_row"] = np.asarray(db1, np.float32)[None, :].astype(bf)          # [1,256]
    w["db2_row"] = np.asarray(db2, np.float32)[None, :].astype(bf)          # [1,128]
    w["dbf_row"] = np.asarray(dbf, np.float32)[None, :].astype(bf)          # [1,1024]
    return w


# ---------------------------------------------------------------- device program

def build_program(T, offs, TT):
    """T: [BPC,2] tiles per (block,half); offs: [BPC,2] tile offsets; TT total tiles."""
    Tmax = int(T.max())
    nc = bacc.Bacc(None, target_bir_lowering=False, num_swdge_queues=4)

    # ---- I/O
    xsA_d = nc.dram_tensor("xsA", [NC * LOA, 128], BF16, kind="ExternalInput")
    xsB_d = nc.dram_tensor("xsB", [NC * HIB, 128], BF16, kind="ExternalInput")
    xloc_d = nc.dram_tensor("xloc", [NPC, 128], BF16, kind="ExternalInput")
    idx_d = nc.dram_tensor("idxs", [128, TT * 8], I16, kind="ExternalInput")
    dloc_d = nc.dram_tensor("dstloc", [128, TT], BF16, kind="ExternalInput")
    gcount_d = nc.dram_tensor("gcount", [1, 2 * BPC], mybir.dt.int32, kind="ExternalInput")
    sqd_d = nc.dram_tensor("sqrtdeg_row", [1, NPC], BF16, kind="ExternalInput")
    dinv_d = nc.dram_tensor("dinv_col", [128, BPC], F32, kind="ExternalInput")
    dinv2_d = nc.dram_tensor("dinv2_col", [128, BPC], F32, kind="ExternalInput")
    dinvb_d = nc.dram_tensor("dinvb", [128, NPC], BF16, kind="ExternalInput")
    one_d = nc.dram_tensor("one_row", [1, 128], BF16, kind="ExternalInput")
    R_d = nc.dram_tensor("Rbig", [128, Tmax * 128], BF16, kind="ExternalInput")
    id_d = nc.dram_tensor("ident", [128, 128], BF16, kind="ExternalInput")
    wnames = {"eW1": [128, 128], "eW2p": [128, 128], "eWfp": [128, 128],
              "dW1p": [128, 256], "dW2": [256, 128], "dWf": [128, 1024],
              "eb1_row": [1, 128], "eb2p_row": [1, 128], "ebf_row": [1, 128],
              "db1_row": [1, 256], "db2_row": [1, 128], "dbf_row": [1, 1024]}
    w_d = {k: nc.dram_tensor(k, shp, BF16, kind="ExternalInput")
           for k, shp in wnames.items()}
    out_d = nc.dram_tensor("xhat", [2, NPC, 512], BF16, kind="ExternalOutput")

    with tile.TileContext(nc) as tc:
        with tc.tile_pool(name="const", bufs=1) as cpool, \
             tc.tile_pool(name="acts", bufs=1) as apool, \
             tc.tile_pool(name="dram", bufs=1, space="DRAM") as dram, \
             tc.tile_pool(name="wps", bufs=4, space="PSUM") as pps, \
             tc.tile_pool(name="wtr", bufs=4, space="PSUM") as ptr, \
             tc.tile_pool(name="wm", bufs=8) as pm, \
             tc.tile_pool(name="ws", bufs=4) as psl, \
             tc.tile_pool(name="wh", bufs=3) as ph, \
             tc.tile_pool(name="wn", bufs=4) as phn:
            nc.gpsimd.load_library(mlp)

            # ---- persistent SBUF state
            w_sb = {}
            for k, shp in wnames.items():
                if shp[0] > 128:
                    continue
                t = cpool.tile(shp, BF16, name=f"w_{k}")
                nc.sync.dma_start(t[:], w_d[k][:])
                w_sb[k] = t
            dW2a = cpool.tile([128, 128], BF16, name="w_dW2a")
            nc.sync.dma_start(dW2a[:], w_d["dW2"][0:128, :])
            dW2b = cpool.tile([128, 128], BF16, name="w_dW2b")
            nc.sync.dma_start(dW2b[:], w_d["dW2"][128:256, :])
            idx_sb = cpool.tile([128, TT * 8], I16, name="idx_sb")
            nc.sync.dma_start(idx_sb[:], idx_d[:])
            dloc_sb = cpool.tile([128, TT], BF16, name="dloc_sb")
            nc.sync.dma_start(dloc_sb[:], dloc_d[:])
            gcount_sb = cpool.tile([1, 2 * BPC], mybir.dt.int32, name="gcount_sb")
            nc.sync.dma_start(gcount_sb[:], gcount_d[:])
            sqd_sb = cpool.tile([1, NPC], BF16, name="sqd_sb")
            nc.sync.dma_start(sqd_sb[:], sqd_d[:])
            dinv_sb = cpool.tile([128, BPC], F32, name="dinv_sb")
            nc.sync.dma_start(dinv_sb[:], dinv_d[:])
            dinv2_sb = cpool.tile([128, BPC], F32, name="dinv2_sb")
            nc.sync.dma_start(dinv2_sb[:], dinv2_d[:])
            dinvb_sb = cpool.tile([128, NPC], BF16, name="dinvb_sb")
            nc.sync.dma_start(dinvb_sb[:], dinvb_d[:])
            one_sb = cpool.tile([1, 128], BF16, name="one_sb")
            nc.sync.dma_start(one_sb[:], one_d[:])
            R_sb = cpool.tile([128, Tmax * 128], BF16, name="R_sb")
            nc.sync.dma_start(R_sb[:], R_d[:])
            id_sb = cpool.tile([128, 128], BF16, name="id_sb")
            nc.sync.dma_start(id_sb[:], id_d[:])

            R3 = R_sb[:].rearrange("p (t d) -> p t d", d=128)

            uT = {}  # feature-major activation arrays per stage
            for nm in ("a1", "u2", "w2", "a3", "u4a", "u4b", "f"):
                uT[nm] = apool.tile([128, NPC], BF16, name=f"{nm}T")

            qstate = [0]

            # ---------------- SpMM: gather-aggregate into per-block PSUM
            # chains; pass A includes the self-loop identity mm (+ optional
            # dst-side sqd x bias mm); pass B re-injects the bf16 pass-A
            # partial via an identity matmul; per-block copy-out via copy_cb.
            def spmm(layer, bufA, bufB, locA, locB, bias_row, out_t, copy_cb,
                     epi=None):
                for hf in range(2):
                    buf = bufA if hf == 0 else bufB
                    for b in range(BPC):
                        Tb = int(T[b, hf]); off = int(offs[b, hf])
                        pb = pps.tile([128, 128], F32, tag="pb")
                        if hf == 0:
                            hblk = ph.tile([128, 128], BF16, tag="hblk")
                            if b < LOB:
                                nc.sync.dma_start(hblk[:], locA[b * 128:(b + 1) * 128, :])
                            else:
                                nc.sync.dma_start(hblk[:], locB[(b - LOB) * 128:(b - LOB + 1) * 128, :])
                            if bias_row is not None:
                                nc.tensor.matmul(
                                    pb[:], bias_row[0:1, :],
                                    sqd_sb[0:1, b * 128:(b + 1) * 128],
                                    start=True, stop=False)
                            nc.tensor.matmul(
                                pb[:], hblk[:], id_sb[:],
                                start=(bias_row is None), stop=False)
                        else:
                            nc.tensor.matmul(
                                pb[:], id_sb[:], out_t[:, b * 128:(b + 1) * 128],
                                start=True, stop=False)
                        msg = pm.tile([128, Tmax, 128], BF16, tag="msg")
                        if layer == 1 and hf == 0 and b < 8:
                            nc.vector.memset(msg[:], 0.0)
                        creg = nc.gpsimd.alloc_register()
                        nc.gpsimd.load(creg, gcount_sb[0:1, hf * BPC + b:hf * BPC + b + 1])
                        nc.gpsimd.dma_gather(
                            msg[:, :Tb, :], buf[:],
                            idx_sb[:, off * 8:(off + Tb) * 8],
                            Tb * 128, creg, 128, single_packet=False,
                            queue_num=qstate[0])
                        qstate[0] = (qstate[0] + 1) % 4
                        S = psl.tile([128, Tmax, 128], BF16, tag="S")
                        d3 = dloc_sb[:, off:off + Tb].broadcast_to([128, Tb, 128])
                        nc.vector.tensor_tensor(S[:, :Tb, :], R3[:, :Tb, :], d3,
                                                mybir.AluOpType.is_equal)
                        for t in range(Tb):
                            nc.tensor.matmul(
                                pb[:], msg[:, t, :], S[:, t, :],
                                start=False, stop=(t == Tb - 1))
                        osl = out_t[:, b * 128:(b + 1) * 128]
                        copy_cb(b, hf, pb, osl)
                        if epi is not None and hf == 1:
                            epi(b)

            def cb_plain(b, hf, pb, osl):
                nc.scalar.activation(osl, pb[:], mybir.ActivationFunctionType.Copy)

            def cb_dinvb(b, hf, pb, osl):
                if hf == 0:
                    nc.scalar.activation(osl, pb[:],
                                         mybir.ActivationFunctionType.Copy)
                else:
                    nc.vector.tensor_tensor(
                        osl, pb[:], dinvb_sb[:, b * 128:(b + 1) * 128],
                        mybir.AluOpType.mult)

            def cb_final(b, hf, pb, osl):
                nc.scalar.activation(osl, pb[:], mybir.ActivationFunctionType.Copy)
                if hf == 0:
                    return
                # final stage for block b: xhat = d*(f dWf + s x dbf), bf16 out
                for cb in range(2):
                    pf = ptr.tile([128, 512], F32, tag="tr")
                    nc.tensor.matmul(pf[:], osl,
                                     w_sb["dWf"][:, cb * 512:(cb + 1) * 512],
                                     start=True, stop=False)
                    nc.tensor.matmul(pf[:],
                                     sqd_sb[0:1, b * 128:(b + 1) * 128],
                                     w_sb["dbf_row"][0:1, cb * 512:(cb + 1) * 512],
                                     start=False, stop=True)
                    ob = phn.tile([128, 512], BF16, tag="ob")
                    nc.scalar.activation(ob[:], pf[:],
                                         mybir.ActivationFunctionType.Copy,
                                         scale=dinv_sb[:, b:b + 1])
                    nc.sync.dma_start(out_d[cb, b * 128:(b + 1) * 128, :], ob[:])

            # ---------------- feature-major "lite" transform, one 128-node tile:
            # out_fm = act(W^T @ in_fm + bias x ones)
            def tlite_tile(nt, in_t, Ws, bias_row, out_ts, act):
                for chb in range(len(out_ts)):
                    pt = ptr.tile([128, 128], F32, tag="tr")
                    nc.tensor.matmul(pt[:], Ws[:, chb * 128:(chb + 1) * 128],
                                     in_t[:, nt * 128:(nt + 1) * 128],
                                     start=True, stop=False)
                    nc.tensor.matmul(pt[:], bias_row[0:1, chb * 128:(chb + 1) * 128],
                                     one_sb[0:1, :], start=False, stop=True)
                    nc.scalar.activation(
                        out_ts[chb][:, nt * 128:(nt + 1) * 128], pt[:], act)

            # ---------------- node-major transform + shard write, one tile:
            # shard rows = scale_col * (sum_k u_k^T @ W_k [+ s x bias])
            def transform_tile(nt, parts, bias_row, shards, scale_col):
                shA, shB = shards
                hb = ptr.tile([128, 128], F32, tag="tr")
                for ki, (ut, Wk) in enumerate(parts):
                    nc.tensor.matmul(hb[:], ut[:, nt * 128:(nt + 1) * 128],
                                     Wk[:], start=(ki == 0),
                                     stop=(bias_row is None and
                                           ki == len(parts) - 1))
                if bias_row is not None:
                    nc.tensor.matmul(hb[:], sqd_sb[0:1, nt * 128:(nt + 1) * 128],
                                     bias_row[0:1, :], start=False, stop=True)
                hn = phn.tile([128, 128], BF16, tag="hn")
                nc.scalar.activation(hn[:], hb[:],
                                     mybir.ActivationFunctionType.Copy,
                                     scale=scale_col[:, nt:nt + 1])
                if nt < LOB:
                    nc.sync.dma_start(shA[nt * 128:(nt + 1) * 128, :], hn[:])
                else:
                    nc.sync.dma_start(shB[(nt - LOB) * 128:(nt - LOB + 1) * 128, :], hn[:])

            def mkshard(name, ch):
                sA = dram.tile([LOA, ch], BF16, name=f"{name}_shardA")
                sB = dram.tile([HIB, ch], BF16, name=f"{name}_shardB")
                fA = dram.tile([NC * LOA, ch], BF16, name=f"{name}_fullA", addr_space="Shared")
                fB = dram.tile([NC * HIB, ch], BF16, name=f"{name}_fullB", addr_space="Shared")
                return sA, sB, fA, fB

            def allgather(sX, fX):
                nc.gpsimd.collective_compute(
                    "AllGather", mybir.AluOpType.bypass,
                    replica_groups=[list(range(NC))],
                    ins=[sX.opt()], outs=[fX.opt()])

            # ================= network =================
            h2sA, h2sB, h2fA, h2fB = mkshard("h2", 128)
            h3sA, h3sB, h3fA, h3fB = mkshard("h3", 128)
            h4sA, h4sB, h4fA, h4fB = mkshard("h4", 128)

            relu_act = mybir.ActivationFunctionType.Relu

            # L1: gather host-staged xs; a1 = d*(sum + self); per-block
            # epilogue: u2 tile = relu(eW1^T a1 + eb1) == h1, then
            # h2' tile = d*(u2 eW2p) -> shard
            def epi1(b):
                tlite_tile(b, uT["a1"], w_sb["eW1"], w_sb["eb1_row"],
                           [uT["u2"]], relu_act)
                transform_tile(b, [(uT["u2"], w_sb["eW2p"])], None,
                               (h2sA[:], h2sB[:]), dinv_sb)

            spmm(1, xsA_d, xsB_d, xloc_d[0:LOA, :], xloc_d[LOA:NPC, :],
                 None, uT["a1"], cb_dinvb, epi=epi1)
            allgather(h2sA, h2fA)
            allgather(h2sB, h2fB)

            # L2 spmm (+eb2p); epilogue: zrow tile = d^2*(w2 eWfp + s x ebf)
            def epi2(b):
                transform_tile(b, [(uT["w2"], w_sb["eWfp"])], w_sb["ebf_row"],
                               (h3sA[:], h3sB[:]), dinv2_sb)

            spmm(2, h2fA, h2fB, h2sA[:], h2sB[:], w_sb["eb2p_row"], uT["w2"],
                 cb_plain, epi=epi2)
            allgather(h3sA, h3fA)
            allgather(h3sB, h3fB)

            # L3: a3 = d*(sum zrow + self); epilogue: u4 = relu(dW1p^T a3 +
            # db1) == h3, then h4' tile = d*(u4 dW2) -> shard
            def epi3(b):
                tlite_tile(b, uT["a3"], w_sb["dW1p"], w_sb["db1_row"],
                           [uT["u4a"], uT["u4b"]], relu_act)
                transform_tile(b, [(uT["u4a"], dW2a), (uT["u4b"], dW2b)], None,
                               (h4sA[:], h4sB[:]), dinv_sb)

            spmm(3, h3fA, h3fB, h3sA[:], h3sB[:], None, uT["a3"], cb_dinvb,
                 epi=epi3)
            allgather(h4sA, h4fA)
            allgather(h4sB, h4fB)

            # L4 spmm (+db2); final stage emitted per block via cb_final
            spmm(4, h4fA, h4fB, h4sA[:], h4sB[:], w_sb["db2_row"], uT["f"],
                 cb_final)

    nc.finalize()
    return nc


# ---------------------------------------------------------------- entry point

def kernel(x, edge_index, eW1, eb1, eW2, eb2, eWf, ebf,
           dW1, db1, dW2, db2, dWf, dbf):
    x = np.asarray(x, dtype=np.float32)
    edge_index = np.asarray(edge_index)

    pre = preprocess(edge_index)
    w = pad_weights(eW1, eb1, eW2, eb2, eWf, ebf, dW1, db1, dW2, db2, dWf, dbf)
    in_maps = make_inmaps(pre, x, w)
    nc = build_program(pre["T"], pre["offs"], pre["TT"])

    trace = os.environ.get("GCAE_TRACE", "0") == "1"
    if trace:
        trace = _install_profile_hook()
    res = None
    last_err = None
    for attempt in range(3):
        try:
            res = run_bass_kernel_spmd(nc, in_maps, core_ids=list(range(NC)),
                                       trace=trace and attempt == 0)
            break
        except Exception as e:  # transient device wedge: retry, drop tracing
            last_err = e
    if res is None:
        raise last_err
    if trace and res.exec_time_ns:
        print(f"HW exec time: {res.exec_time_ns} ns")

    xhat_pad = np.empty((NPAD, 1024), dtype=np.float32)
    for c in range(NC):
        o = np.asarray(res.results[c]["xhat"]).astype(np.float32)
        xhat_pad[c * NPC:(c + 1) * NPC, 0:512] = o[0]
        xhat_pad[c * NPC:(c + 1) * NPC, 512:1024] = o[1]
    return xhat_pad[pre["node_new"]]
